# revision 27
# baseline (speedup 1.0000x reference)
"""Trainium2 Bass kernel for nn_CNNPredictor (attention scorer + CNN head).

Sharding: data-parallel over batch b (8 batches -> 8 NeuronCores), no
collectives. Each core computes its batch's [TYPE_NUM] output row; host
gathers to [B, TYPE_NUM].

Math (per batch):
  pre[c,t,:] = [q|ctx|, |q-ctx|, q*ctx] @ W_h.T + b_h   (4e = 1024 hidden)
split as
  pre = A[c] + B[t] + W3 @ |q-ctx| + W4 @ (q*ctx)
with A = q @ W1.T, B = ctx @ W2.T + b_h (both tiny, computed in fp8
DoubleRow matmuls). The big K=512 contraction runs in fp8 DoubleRow
(2 matmuls per 128-wide hidden chunk instead of 4 bf16 ones); the A/B
bias is folded in with ONE k=72 bf16 indicator matmul per chunk whose
stationary stacks A rows (c) over B rows (t of this tile). tanh outputs
are fp8 so the W_v reduction also runs DoubleRow (4 matmuls per tile).
W1..W4, b_h, W_v are pre-scaled x64 on the host so fp8 weights stay in
the normal range; the tanh activation un-scales with scale=1/64 and the
score copy un-scales the W_v x64. Only t-positions with mask==1 are
computed (padded to a multiple of 8); masked softmax handles padding.
Softmax + second pass + CNN head stay bf16 (fp8 there breaks the 2e-2
error budget; measured on CPU).
"""

import os
import sys

for _p in ("/opt/trn_rl_repo",):
    if _p not in sys.path:
        sys.path.append(_p)

import numpy as np
from ml_dtypes import bfloat16, float8_e4m3

import concourse.bass as bass
import concourse.bacc as bacc
import concourse.tile as tile
from concourse import mybir
from concourse.bass_utils import run_bass_kernel_spmd
from concourse.bass_interp import get_hw_module

F32 = mybir.dt.float32
BF16 = mybir.dt.bfloat16
FP8 = mybir.dt.float8e4
AF = mybir.ActivationFunctionType
ALU = mybir.AluOpType
DR = mybir.MatmulPerfMode.DoubleRow

B, C, T, E = 8, 64, 128, 256
H = 4 * E  # 1024
NF, TYPE_NUM = 128, 40
KS = (5, 4, 3)
NEG = -1e10
NUM_CORES = 8
WS = 64.0  # host-side scale on W1..W4/bh/Wv so fp8 weights are ~N(0,1)

# module-level knobs for test harness
TRACE = False
LAST_EXEC_NS = None

_CACHE = {}


def _build_program(n_pad):
    """Build the SPMD Bass program for padded active length n_pad (mult of 8)."""
    stage = int(os.environ.get("KSTAGE", "99"))
    R = n_pad // 8  # number of 512-wide r tiles; r = (t, c) t-major
    n_pad16 = -(-n_pad // 16) * 16  # DoubleRow APs need 16B-aligned dim1 step

    nc = bacc.Bacc("TRN2", target_bir_lowering=False, debug=False,
                   num_devices=NUM_CORES)

    # fp8 attention-path tensors (loaded first; small)
    d_qT8 = nc.dram_tensor("qT8", [128, 2, C], FP8, kind="ExternalInput")
    d_ctxT8 = nc.dram_tensor("ctxT8", [128, 2, n_pad16], FP8,
                             kind="ExternalInput")
    d_W8A = nc.dram_tensor("W8A", [128, 2, H], FP8, kind="ExternalInput")
    d_W8B = nc.dram_tensor("W8B", [128, 2, H], FP8, kind="ExternalInput")
    d_W8C = nc.dram_tensor("W8C", [128, 2, 8, 128], FP8, kind="ExternalInput")
    d_W8D = nc.dram_tensor("W8D", [128, 2, 8, 128], FP8, kind="ExternalInput")
    d_Wv8 = nc.dram_tensor("Wv8", [128, 2, 16], FP8, kind="ExternalInput")
    d_IndAB = nc.dram_tensor("IndAB", [C + 8, 512], BF16, kind="ExternalInput")
    d_bh64 = nc.dram_tensor("bh64", [1, H], BF16, kind="ExternalInput")
    d_qT = nc.dram_tensor("qT", [128, 2, C], BF16, kind="ExternalInput")
    d_ctxT = nc.dram_tensor("ctxT", [128, 2, n_pad], BF16, kind="ExternalInput")
    d_maskadd = nc.dram_tensor("maskadd", [C, n_pad], F32, kind="ExternalInput")
    # bf16 tail tensors
    d_ctx = nc.dram_tensor("ctx", [n_pad, E], BF16, kind="ExternalInput")
    d_WhT = nc.dram_tensor("WhT", [128, 8, H], BF16, kind="ExternalInput")
    d_bh = nc.dram_tensor("bh", [1, H], BF16, kind="ExternalInput")
    d_WlT = nc.dram_tensor("WlT", [128, 8, E], BF16, kind="ExternalInput")
    d_bl = nc.dram_tensor("bl", [128, 2], F32, kind="ExternalInput")
    d_cw = [nc.dram_tensor(f"cw{i}", [128, KS[i], 2, NF], BF16,
                           kind="ExternalInput") for i in range(3)]
    d_bhT = nc.dram_tensor("bhT", [128, 8], F32, kind="ExternalInput")
    d_cbT = nc.dram_tensor("cbT", [128, 3], F32, kind="ExternalInput")
    d_WcT = nc.dram_tensor("WcT", [128, 3, TYPE_NUM], BF16, kind="ExternalInput")
    d_bc = nc.dram_tensor("bc", [TYPE_NUM, 1], F32, kind="ExternalInput")
    d_out = nc.dram_tensor("out", [TYPE_NUM], F32, kind="ExternalOutput")

    with tile.TileContext(nc) as tc:
        with (
            tc.tile_pool(name="const", bufs=1) as cpool,
            tc.tile_pool(name="ft", bufs=3) as ftpool,
            tc.tile_pool(name="th", bufs=6) as thpool,
            tc.tile_pool(name="soft", bufs=1) as spool,
            tc.tile_pool(name="ps_main", bufs=3, space="PSUM") as ps_main,
            tc.tile_pool(name="ps_sm", bufs=2, space="PSUM") as ps_sm,
            tc.tile_pool(name="drp", bufs=1, space="DRAM") as drpool,
        ):
            d_scr = drpool.tile([C, n_pad], F32)
            # ---- load constants -------------------------------------------
            # sync queue: small attention-path tensors (plus per-rt AB
            # DMAs and drip-fed tail weights from the loop). scalar/gpsimd
            # queues carry the fp8 weights in parallel.
            qT8 = cpool.tile([128, 2, C], FP8)
            nc.sync.dma_start(out=qT8[:], in_=d_qT8[:])
            ctxT8 = cpool.tile([128, 2, n_pad16], FP8)
            nc.sync.dma_start(out=ctxT8[:], in_=d_ctxT8[:])
            ctxT = cpool.tile([128, 2, n_pad], BF16)
            nc.sync.dma_start(out=ctxT[:], in_=d_ctxT[:])
            qT = cpool.tile([128, 2, C], BF16)
            nc.sync.dma_start(out=qT[:], in_=d_qT[:])
            bh64 = cpool.tile([1, H], BF16)
            nc.sync.dma_start(out=bh64[:], in_=d_bh64[:])
            IndAB = cpool.tile([C + 8, 512], BF16)
            nc.sync.dma_start(out=IndAB[:], in_=d_IndAB[:])
            maskadd = cpool.tile([C, n_pad], F32)
            nc.sync.dma_start(out=maskadd[:], in_=d_maskadd[:])
            ctxa = cpool.tile([n_pad, E], BF16)
            nc.sync.dma_start(out=ctxa[:], in_=d_ctx[:])
            W8A = cpool.tile([128, 2, H], FP8)
            nc.scalar.dma_start(out=W8A[:, :, 0:512], in_=d_W8A[:, :, 0:512])
            nc.scalar.dma_start(out=W8A[:, :, 512:H], in_=d_W8A[:, :, 512:H])
            W8C = cpool.tile([128, 2, 8, 128], FP8)
            nc.scalar.dma_start(out=W8C[:], in_=d_W8C[:])
            Wv8 = cpool.tile([128, 2, 16], FP8)
            nc.scalar.dma_start(out=Wv8[:], in_=d_Wv8[:])
            W8B = cpool.tile([128, 2, H], FP8)
            nc.gpsimd.dma_start(out=W8B[:, :, 0:512], in_=d_W8B[:, :, 0:512])
            nc.gpsimd.dma_start(out=W8B[:, :, 512:H], in_=d_W8B[:, :, 512:H])
            W8D = cpool.tile([128, 2, 8, 128], FP8)
            nc.gpsimd.dma_start(out=W8D[:], in_=d_W8D[:])

            # tail-only tensors; DMAs drip-fed on sync from inside the loop
            WhT = cpool.tile([128, 8, H], BF16)
            bh = cpool.tile([1, H], BF16)
            WlT = cpool.tile([128, 8, E], BF16)
            bl = cpool.tile([128, 2], F32)
            bhT = cpool.tile([128, 8], F32)
            cbT = cpool.tile([128, 3], F32)
            cw = [cpool.tile([128, KS[i], 2, NF], BF16, tag=f"cw{i}",
                             name=f"cw{i}t") for i in range(3)]
            WcT = cpool.tile([128, 3, TYPE_NUM], BF16)
            bc = cpool.tile([TYPE_NUM, 1], F32)
            tail_dmas = [(WhT[:, kc, :], d_WhT[:, kc, :]) for kc in range(8)]
            tail_dmas += [(bh[:], d_bh[:]), (WlT[:], d_WlT[:]),
                          (bl[:], d_bl[:]), (bhT[:], d_bhT[:]),
                          (cbT[:], d_cbT[:])]
            tail_dmas += [(cw[i][:], d_cw[i][:]) for i in range(3)]
            tail_dmas += [(WcT[:], d_WcT[:]), (bc[:], d_bc[:])]

            ones = cpool.tile([1, max(n_pad, C)], BF16)
            nc.vector.memset(ones[:], 1.0)

            # q broadcast along t: qbc[p, ec, t, c] = qT[p, ec, c]
            qbc = cpool.tile([128, 2, 8, C], BF16)
            nc.vector.tensor_copy(qbc[:, :, 0, :], qT[:])
            nc.vector.tensor_copy(qbc[:, :, 1, :], qbc[:, :, 0, :])
            nc.vector.tensor_copy(qbc[:, :, 2:4, :], qbc[:, :, 0:2, :])
            nc.vector.tensor_copy(qbc[:, :, 4:8, :], qbc[:, :, 0:4, :])

            # PE warm-up burst on junk data: keeps the HAM activity window
            # busy while the first DMAs land so phase 0 runs at full clock.
            junk = cpool.tile([128, 512], BF16)
            nc.vector.memset(junk[:], 0.5)
            psj = ps_sm.tile([128, 512], F32, tag="sm")
            for wi in range(14):
                nc.tensor.matmul(psj[:], junk[:, 0:128], junk[:],
                                 start=(wi == 0), stop=(wi == 13))

            def junk_burst(n):
                # PE-idle bridge: enough matmul activity to stop the HAM
                # clock gate from re-throttling during serial scalar/vector
                # sections. Uses the (then idle) ps_main pool.
                pj = ps_main.tile([128, 2, 512], F32, tag="P")
                for wi in range(n):
                    nc.tensor.matmul(pj[:, 0, :], junk[:, 0:128], junk[:],
                                     start=(wi == 0), stop=(wi == n - 1))

            # ---- phase 0: A = q @ W1.T ; B = ctx @ W2.T + b_h (all x64) ---
            # A rows (c: 0..63) and B rows (t: 64..71, rewritten per rt via
            # SBUF->SBUF DMA) stack into the two alternating AB stationaries.
            AB0 = cpool.tile([C + 8, H], BF16, tag="AB0")
            AB1 = cpool.tile([C + 8, H], BF16, tag="AB1")
            B_T = cpool.tile([n_pad, H], BF16)
            for jn in range(2):
                jsl = slice(jn * 512, (jn + 1) * 512)
                psA = ps_sm.tile([C, 512], F32, tag="sm")
                nc.tensor.matmul(psA[:], qT8[:], W8A[:, :, jsl],
                                 start=True, stop=True, perf_mode=DR)
                nc.scalar.copy(AB0[0:C, jsl], psA[:])
                nc.scalar.copy(AB1[0:C, jsl], psA[:])
                psB = ps_sm.tile([n_pad16, 512], F32, tag="sm")
                nc.tensor.matmul(psB[:], ctxT8[:], W8B[:, :, jsl],
                                 start=True, stop=False, perf_mode=DR)
                nc.tensor.matmul(psB[0:n_pad, :], ones[:, :n_pad],
                                 bh64[:, jsl], start=False, stop=True,
                                 skip_group_check=True)
                nc.scalar.copy(B_T[:, jsl], psB[0:n_pad, :])

            if stage < 2:
                nc.gpsimd.dma_start(out=d_out[:], in_=B_T[0:TYPE_NUM, 0])

            junk_burst(8)  # bridge PE over the phase0 -> rt0 dependency gap

            # ---- phase 1: scores over (c, active t) -----------------------
            scoresT = spool.tile([C, n_pad], F32)
            if stage >= 2:
                ab_tiles = (AB0, AB1)
                tail_fed = 0
                for rt in range(R):
                    AB = ab_tiles[rt % 2]
                    # stationary B rows for this tile -> partitions 64..71
                    nc.sync.dma_start(
                        out=AB[C:C + 8, :],
                        in_=B_T[rt * 8:(rt + 1) * 8, :])
                    if rt >= 2:
                        # drip-feed tail-weight DMAs (3 per rt) on sync
                        for _ in range(3):
                            if tail_fed < len(tail_dmas):
                                o, i_ = tail_dmas[tail_fed]
                                nc.sync.dma_start(out=o, in_=i_)
                                tail_fed += 1
                    # ctx broadcast along c on GpSimd (keeps DVE short):
                    # cbc[p, ec, t, c] = ctxT[p, ec, rt*8+t]
                    cbc = ftpool.tile([128, 2, 8, C], BF16, tag="cbc")
                    nc.gpsimd.tensor_copy(cbc[:, :, :, 0],
                                          ctxT[:, :, rt * 8:(rt + 1) * 8])
                    w = 1
                    while w < C:
                        nc.gpsimd.tensor_copy(cbc[:, :, :, w:2 * w],
                                              cbc[:, :, :, 0:w])
                        w *= 2
                    ftC = ftpool.tile([128, 2, 8, C], FP8, tag="ftC")
                    ftD = ftpool.tile([128, 2, 8, C], FP8, tag="ftD")
                    sc_t = ftpool.tile([128, 2, 8, C], BF16, tag="sc_t")
                    for ec in range(2):
                        bq = qbc[:, ec]
                        bcx = cbc[:, ec]
                        nc.vector.tensor_sub(sc_t[:, ec], bq, bcx)
                        nc.vector.scalar_tensor_tensor(
                            ftC[:, ec], sc_t[:, ec], -1.0, sc_t[:, ec],
                            op0=ALU.mult, op1=ALU.max)
                        nc.vector.tensor_mul(ftD[:, ec], bq, bcx)
                    # MM order per pair keeps fp8-DR matmuls contiguous (a
                    # bf16->DR mode switch costs ~190ns): 4 DR mains + the
                    # previous pair's DR score matmul, then the 2 bf16
                    # indicator matmuls at the end.
                    S = ps_sm.tile([1, 512], F32, tag="sm")
                    ths = []
                    for jp in range(4):  # pairs of 128-wide hidden chunks
                        P2 = ps_main.tile([128, 2, 512], F32, tag="P")
                        TH2 = thpool.tile([128, 2, 512], FP8, tag="TH")
                        jc0, jc1 = jp * 2, jp * 2 + 1
                        jsl0 = slice(jc0 * 128, (jc0 + 1) * 128)
                        jsl1 = slice(jc1 * 128, (jc1 + 1) * 128)
                        nc.tensor.matmul(P2[:, 0, :], W8C[:, :, jc0, :],
                                         ftC[:], start=True, stop=False,
                                         perf_mode=DR)
                        nc.tensor.matmul(P2[:, 0, :], W8D[:, :, jc0, :],
                                         ftD[:], start=False, stop=False,
                                         perf_mode=DR)
                        if jp > 0:
                            nc.tensor.matmul(S[:], Wv8[:, :, jp - 1:jp],
                                             ths[jp - 1][:], start=(jp == 1),
                                             stop=False, perf_mode=DR,
                                             skip_group_check=True)
                        nc.tensor.matmul(P2[:, 1, :], W8C[:, :, jc1, :],
                                         ftC[:], start=True, stop=False,
                                         perf_mode=DR)
                        nc.tensor.matmul(P2[:, 1, :], W8D[:, :, jc1, :],
                                         ftD[:], start=False, stop=False,
                                         perf_mode=DR)
                        nc.tensor.matmul(P2[:, 0, :], AB[:, jsl0], IndAB[:],
                                         start=False, stop=True,
                                         skip_group_check=True)
                        nc.tensor.matmul(P2[:, 1, :], AB[:, jsl1], IndAB[:],
                                         start=False, stop=True,
                                         skip_group_check=True)
                        nc.scalar.activation(TH2[:], P2[:], AF.Tanh,
                                             scale=1.0 / WS)
                        ths.append(TH2)
                    nc.tensor.matmul(S[:], Wv8[:, :, 3:4], ths[3][:],
                                     start=False, stop=True, perf_mode=DR,
                                     skip_group_check=True)
                    S_sb = thpool.tile([1, 512], F32, tag="S_sb")
                    nc.vector.tensor_scalar_mul(S_sb[:], S[:], 1.0 / WS)
                    # transposed write: d_scr[c, t] (strided, off critical
                    # path) so the final gather is a fast contiguous read
                    nc.scalar.dma_start(
                        out=d_scr[:, rt * 8:(rt + 1) * 8]
                        .rearrange("c t -> t c").unsqueeze(0),
                        in_=S_sb[0:1, :].rearrange("p (t c) -> p t c", c=C))
                while tail_fed < len(tail_dmas):
                    o, i_ = tail_dmas[tail_fed]
                    nc.sync.dma_start(out=o, in_=i_)
                    tail_fed += 1
                nc.scalar.dma_start(out=scoresT[:], in_=d_scr[:])
            if stage == 2:
                nc.sync.dma_start(out=d_out[:], in_=scoresT[0:TYPE_NUM, 0])

            # ---- masked softmax + g = attn @ ctx --------------------------
            if stage >= 3:
                # scores are O(1) so exp() is safe without max-subtraction;
                # masked positions are -1e10 -> exp = 0.
                junk_burst(8)
                nc.vector.tensor_add(scoresT[:], scoresT[:], maskadd[:])
                ex = spool.tile([C, n_pad], F32)
                se = spool.tile([C, 1], F32)
                nc.scalar.activation(ex[:], scoresT[:], AF.Exp,
                                     scale=1.0, accum_out=se[:])
                rse = spool.tile([C, 1], F32)
                nc.vector.reciprocal(rse[:], se[:])
                attn = spool.tile([C, n_pad], BF16)
                nc.vector.tensor_scalar_mul(attn[:], ex[:], rse[:])

                attnT_ps = ps_sm.tile([n_pad, C], BF16, tag="sm")
                nc.tensor.transpose(attnT_ps[:], attn[:], IndAB[0:C, :C])
                attnT = spool.tile([n_pad, C], BF16)
                nc.vector.tensor_copy(attnT[:], attnT_ps[:])
                junk_burst(4)
                # gT[e, c] = (ctx.T @ attn.T)[e, c] -- direct, no transposes
                gT = spool.tile([128, 2, C], BF16)
                for ec in range(2):
                    gT_ps = ps_sm.tile([128, C], F32, tag="sm")
                    nc.tensor.matmul(gT_ps[:],
                                     ctxa[:, ec * 128:(ec + 1) * 128],
                                     attnT[:], start=True, stop=True)
                    nc.scalar.copy(gT[:, ec, :], gT_ps[:])
            if stage == 3:
                nc.gpsimd.dma_start(out=d_out[:], in_=gT[0:TYPE_NUM, 0, 0])

            # ---- phase 2: h2 = tanh([q|g|,|q-g|,q*g] @ Wh.T + bh) ---------
            if stage >= 4:
                junk_burst(4)
                f2C = spool.tile([128, 2, C], BF16)
                f2D = spool.tile([128, 2, C], BF16)
                for ec in range(2):
                    nc.vector.tensor_sub(f2C[:, ec], qT[:, ec, :], gT[:, ec, :])
                    nc.vector.scalar_tensor_tensor(
                        f2C[:, ec], f2C[:, ec], -1.0, f2C[:, ec],
                        op0=ALU.mult, op1=ALU.max)
                    nc.vector.tensor_mul(f2D[:, ec], qT[:, ec, :], gT[:, ec, :])
                h2T = spool.tile([128, 8, C], BF16)
                for jc in range(8):
                    jsl = slice(jc * 128, (jc + 1) * 128)
                    H2 = ps_sm.tile([128, C], F32, tag="sm")
                    for mi, rhs_t in enumerate((qT[:, 0, :], qT[:, 1, :],
                                                gT[:, 0, :], gT[:, 1, :],
                                                f2C[:, 0, :], f2C[:, 1, :],
                                                f2D[:, 0, :], f2D[:, 1, :])):
                        nc.tensor.matmul(H2[:], WhT[:, mi, jsl], rhs_t,
                                         start=(mi == 0), stop=(mi == 7))
                    nc.scalar.activation(h2T[:, jc, :], H2[:], AF.Tanh,
                                         bias=bhT[:, jc:jc + 1], scale=1.0)

                # x.T = W_lin @ h2 : [e, c], e-major for the convs
                xT = spool.tile([128, 2, C], BF16)
                for ec2 in range(2):
                    X = ps_sm.tile([128, C], F32, tag="sm")
                    for jc in range(8):
                        nc.tensor.matmul(
                            X[:], WlT[:, jc, ec2 * 128:(ec2 + 1) * 128],
                            h2T[:, jc, :], start=(jc == 0), stop=(jc == 7))
                    nc.scalar.activation(xT[:, ec2, :], X[:], AF.Identity,
                                         bias=bl[:, ec2:ec2 + 1], scale=1.0)

                # convs + maxpool; conv bias commutes with max over
                # positions, so it folds into the relu bias afterwards
                pooled_raw = spool.tile([NF, 3], F32)
                for i in range(3):
                    ki = KS[i]
                    oi = C - ki + 1
                    Y = ps_sm.tile([NF, oi], F32, tag="sm")
                    nmm = 2 * ki
                    mm = 0
                    for dk in range(ki):
                        for ec2 in range(2):
                            nc.tensor.matmul(Y[:], cw[i][:, dk, ec2, :],
                                             xT[:, ec2, dk:dk + oi],
                                             start=(mm == 0),
                                             stop=(mm == nmm - 1))
                            mm += 1
                    nc.vector.tensor_reduce(pooled_raw[:, i:i + 1], Y[:],
                                            axis=mybir.AxisListType.X,
                                            op=ALU.max)
                pooled = spool.tile([NF, 3], BF16)
                for i in range(3):
                    nc.scalar.activation(pooled[:, i:i + 1],
                                         pooled_raw[:, i:i + 1], AF.Relu,
                                         bias=cbT[:, i:i + 1], scale=1.0)

                # final linear: out = W_cnn @ cnn + b_cnn
                O = ps_sm.tile([TYPE_NUM, 1], F32, tag="sm")
                for i in range(3):
                    nc.tensor.matmul(O[:], WcT[:, i, :], pooled[:, i:i + 1],
                                     start=(i == 0), stop=(i == 2))
                out_sb = spool.tile([TYPE_NUM, 1], F32)
                nc.scalar.activation(out_sb[:], O[:], AF.Identity, bias=bc[:],
                                     scale=1.0)
                nc.sync.dma_start(out=d_out[:], in_=out_sb[:, 0])

    nc.compile()
    nc.m = get_hw_module(nc.m)
    return nc


def _prep_inputs(query, context, mask, W_hidden, b_hidden, W_v, b_v,
                 W_lin, b_lin, conv_w0, conv_b0, conv_w1, conv_b1,
                 conv_w2, conv_b2, W_cnn, b_cnn):
    """Host-side layout prep. Returns (n_pad, per_core_maps)."""
    f32 = np.float32
    mask = np.asarray(mask)
    n_act = mask.sum(1)
    if n_act.min() == 0:
        # degenerate: keep every position, mask on device via maskadd
        idxs = [np.arange(T) for _ in range(B)]
        n_pad = T
        mads = [np.where(mask[b] < 1, NEG, 0.0).astype(f32) for b in range(B)]
    else:
        n_pad = max(8, int(-(-int(n_act.max()) // 8) * 8))
        idxs, mads = [], []
        for b in range(B):
            idx = np.nonzero(mask[b])[0]
            ma = np.full(n_pad, NEG, f32)
            ma[:len(idx)] = 0.0
            idx = np.concatenate([idx, np.zeros(n_pad - len(idx), np.int64)])
            idxs.append(idx)
            mads.append(ma)
    n_pad16 = -(-n_pad // 16) * 16

    bf = bfloat16
    f8 = float8_e4m3
    Wh = np.asarray(W_hidden, f32)
    # W chunk c (of 8) = rows/cols [c*128:(c+1)*128] of the k=1024 dim.
    # WhT[p, kc, j] = Wh[j, kc*128 + p]
    WhT = np.ascontiguousarray(Wh.T).reshape(8, 128, H).transpose(1, 0, 2)
    WhTs = WhT * WS
    # fp8 pair tensors: [ki, pair(2), ...]
    W8A = np.ascontiguousarray(WhTs[:, 0:2, :])            # k chunks 0,1 (q)
    W8B = np.ascontiguousarray(WhTs[:, 2:4, :])            # k chunks 2,3 (ctx)
    W8C = np.ascontiguousarray(
        WhTs[:, 4:6, :].reshape(128, 2, 8, 128))           # |q-ctx|
    W8D = np.ascontiguousarray(
        WhTs[:, 6:8, :].reshape(128, 2, 8, 128))           # q*ctx
    Wv8 = np.zeros((128, 2, 16), f32)
    Wv8[:, :, 0:4] = (np.asarray(W_v, f32)[0] * WS).reshape(4, 2, 128) \
        .transpose(2, 1, 0)

    IndAB = np.concatenate([
        np.tile(np.eye(C, dtype=f32), (1, 8)),
        np.kron(np.eye(8, dtype=f32), np.ones((1, C), f32)),
    ], axis=0)

    query = np.asarray(query, f32)
    qTf = np.ascontiguousarray(query.T.reshape(2, 128, C).transpose(1, 0, 2))
    shared = {
        "qT": qTf.astype(bf),
        "qT8": qTf.astype(f8),
        "W8A": W8A.astype(f8),
        "W8B": W8B.astype(f8),
        "W8C": W8C.astype(f8),
        "W8D": W8D.astype(f8),
        "Wv8": Wv8.astype(f8),
        "IndAB": IndAB.astype(bf),
        "bh64": (np.asarray(b_hidden, f32) * WS).reshape(1, H).astype(bf),
        "WhT": np.ascontiguousarray(WhT).astype(bf),
        "bh": np.asarray(b_hidden, f32).reshape(1, H).astype(bf),
        "WlT": np.ascontiguousarray(
            np.asarray(W_lin, f32).T.reshape(8, 128, E).transpose(1, 0, 2)
        ).astype(bf),
        "bl": np.ascontiguousarray(
            np.asarray(b_lin, f32).reshape(2, 128).T).astype(f32),
        "bhT": np.ascontiguousarray(
            np.asarray(b_hidden, f32).reshape(8, 128).T).astype(f32),
        "cbT": np.stack([np.asarray(x, f32) for x in
                         (conv_b0, conv_b1, conv_b2)], axis=1).astype(f32),
        "WcT": np.ascontiguousarray(
            np.asarray(W_cnn, f32).T.reshape(3, 128, TYPE_NUM)
            .transpose(1, 0, 2)).astype(bf),
        "bc": np.asarray(b_cnn, f32).reshape(TYPE_NUM, 1).astype(f32),
    }
    for i, w in enumerate((conv_w0, conv_w1, conv_w2)):
        w = np.asarray(w, f32)  # [NF, E, ki]
        arr = w.transpose(1, 2, 0).reshape(2, 128, KS[i], NF) \
            .transpose(1, 2, 0, 3)  # [128, ki, 2, NF]
        shared[f"cw{i}"] = np.ascontiguousarray(arr).astype(bf)

    context = np.asarray(context, f32)
    per_core = []
    for b in range(B):
        ctx_act = context[b][idxs[b]]  # [n_pad, E]
        ctx_act = ctx_act * (mads[b] == 0.0)[:, None]  # zero padded rows
        ctxT = np.ascontiguousarray(
            ctx_act.T.reshape(2, 128, n_pad).transpose(1, 0, 2))
        ctxT8 = np.zeros((128, 2, n_pad16), f32)
        ctxT8[:, :, :n_pad] = ctxT
        per_core.append({
            "ctx": np.ascontiguousarray(ctx_act).astype(bf),
            "ctxT": ctxT.astype(bf),
            "ctxT8": ctxT8.astype(f8),
            "maskadd": np.tile(mads[b][None, :], (C, 1)).astype(f32),
            **shared,
        })
    return n_pad, per_core


def kernel(**inputs):
    global LAST_EXEC_NS
    n_pad, per_core = _prep_inputs(**inputs)
    key = (n_pad, os.environ.get("KSTAGE", "99"))
    if key not in _CACHE:
        _CACHE[key] = _build_program(n_pad)
    nc = _CACHE[key]
    res = run_bass_kernel_spmd(nc, per_core, list(range(NUM_CORES)),
                               trace=TRACE)
    LAST_EXEC_NS = res.exec_time_ns
    out = np.stack([res.results[i]["out"] for i in range(NUM_CORES)])
    return out.astype(np.float32)


# revision 30
# speedup vs baseline: 1.2358x; 1.2358x over previous
"""Trainium2 Bass kernel for nn_CNNPredictor (attention scorer + CNN head).

Sharding: data-parallel over batch b (8 batches -> 8 NeuronCores), no
collectives. Each core computes its batch's [TYPE_NUM] output row; host
gathers to [B, TYPE_NUM].

Math (per batch):
  pre[c,t,:] = [q|ctx|, |q-ctx|, q*ctx] @ W_h.T + b_h   (4e = 1024 hidden)
split as
  pre = A[c] + B[t] + W3 @ |q-ctx| + W4 @ (q*ctx)
with A = q @ W1.T, B = ctx @ W2.T + b_h (both tiny, computed in fp8
DoubleRow matmuls). The big K=512 contraction runs in fp8 DoubleRow
(2 matmuls per 128-wide hidden chunk instead of 4 bf16 ones); the A/B
bias is folded in with ONE k=72 bf16 indicator matmul per chunk whose
stationary stacks A rows (c) over B rows (t of this tile). tanh outputs
are fp8 so the W_v reduction also runs DoubleRow (4 matmuls per tile).
W1..W4, b_h, W_v are pre-scaled x64 on the host so fp8 weights stay in
the normal range; the tanh activation un-scales with scale=1/64 and the
score copy un-scales the W_v x64. Only t-positions with mask==1 are
computed (padded to a multiple of 8); masked softmax handles padding.
Softmax + second pass + CNN head stay bf16 (fp8 there breaks the 2e-2
error budget; measured on CPU).
"""

import os
import sys

for _p in ("/opt/trn_rl_repo",):
    if _p not in sys.path:
        sys.path.append(_p)

import numpy as np
from ml_dtypes import bfloat16, float8_e4m3

import concourse.bass as bass
import concourse.bacc as bacc
import concourse.tile as tile
from concourse import mybir
from concourse.bass_utils import run_bass_kernel_spmd
from concourse.bass_interp import get_hw_module

F32 = mybir.dt.float32
BF16 = mybir.dt.bfloat16
FP8 = mybir.dt.float8e4
AF = mybir.ActivationFunctionType
ALU = mybir.AluOpType
DR = mybir.MatmulPerfMode.DoubleRow

B, C, T, E = 8, 64, 128, 256
H = 4 * E  # 1024
NF, TYPE_NUM = 128, 40
KS = (5, 4, 3)
NEG = -1e10
NUM_CORES = 8
WS = 64.0  # host-side scale on W1..W4/bh/Wv so fp8 weights are ~N(0,1)

# module-level knobs for test harness
TRACE = False
LAST_EXEC_NS = None

_CACHE = {}


def _build_program(n_pad):
    """Build the SPMD Bass program for padded active length n_pad (mult of 8)."""
    stage = int(os.environ.get("KSTAGE", "99"))
    R = n_pad // 8  # number of 512-wide r tiles; r = (t, c) t-major
    n_pad16 = -(-n_pad // 16) * 16  # DoubleRow APs need 16B-aligned dim1 step

    nc = bacc.Bacc("TRN2", target_bir_lowering=False, debug=False,
                   num_devices=NUM_CORES)

    # fp8 attention-path tensors (loaded first; small)
    d_qT8 = nc.dram_tensor("qT8", [128, 2, C], FP8, kind="ExternalInput")
    d_ctxT8 = nc.dram_tensor("ctxT8", [128, 2, n_pad16], FP8,
                             kind="ExternalInput")
    d_W8A = nc.dram_tensor("W8A", [128, 2, H], FP8, kind="ExternalInput")
    d_W8B = nc.dram_tensor("W8B", [128, 2, H], FP8, kind="ExternalInput")
    d_W8C = nc.dram_tensor("W8C", [128, 2, 8, 128], FP8, kind="ExternalInput")
    d_W8D = nc.dram_tensor("W8D", [128, 2, 8, 128], FP8, kind="ExternalInput")
    d_Wv8 = nc.dram_tensor("Wv8", [128, 2, 16], FP8, kind="ExternalInput")
    d_IndAB = nc.dram_tensor("IndAB", [C + 8, 512], BF16, kind="ExternalInput")
    d_bh64 = nc.dram_tensor("bh64", [1, H], BF16, kind="ExternalInput")
    d_qT = nc.dram_tensor("qT", [128, 2, C], BF16, kind="ExternalInput")
    d_ctxT = nc.dram_tensor("ctxT", [128, 2, n_pad], BF16, kind="ExternalInput")
    d_maskadd = nc.dram_tensor("maskadd", [C, n_pad], F32, kind="ExternalInput")
    # bf16 tail tensors
    d_ctx = nc.dram_tensor("ctx", [n_pad, E], BF16, kind="ExternalInput")
    d_WhT = nc.dram_tensor("WhT", [128, 8, H], BF16, kind="ExternalInput")
    d_bh = nc.dram_tensor("bh", [1, H], BF16, kind="ExternalInput")
    d_WlT = nc.dram_tensor("WlT", [128, 8, E], BF16, kind="ExternalInput")
    d_bl = nc.dram_tensor("bl", [128, 2], F32, kind="ExternalInput")
    d_cw = [nc.dram_tensor(f"cw{i}", [128, KS[i], 2, NF], BF16,
                           kind="ExternalInput") for i in range(3)]
    d_bhT = nc.dram_tensor("bhT", [128, 8], F32, kind="ExternalInput")
    d_cbT = nc.dram_tensor("cbT", [128, 3], F32, kind="ExternalInput")
    d_WcT = nc.dram_tensor("WcT", [128, 3, TYPE_NUM], BF16, kind="ExternalInput")
    d_bc = nc.dram_tensor("bc", [TYPE_NUM, 1], F32, kind="ExternalInput")
    d_out = nc.dram_tensor("out", [TYPE_NUM], F32, kind="ExternalOutput")

    with tile.TileContext(nc) as tc:
        with (
            tc.tile_pool(name="const", bufs=1) as cpool,
            tc.tile_pool(name="ft", bufs=3) as ftpool,
            tc.tile_pool(name="th", bufs=6) as thpool,
            tc.tile_pool(name="soft", bufs=1) as spool,
            tc.tile_pool(name="ps_main", bufs=3, space="PSUM") as ps_main,
            tc.tile_pool(name="ps_sm", bufs=2, space="PSUM") as ps_sm,
            tc.tile_pool(name="drp", bufs=1, space="DRAM") as drpool,
        ):
            d_scr = drpool.tile([C, n_pad], F32)
            # ---- load constants -------------------------------------------
            # sync queue: small attention-path tensors (plus per-rt AB
            # DMAs and drip-fed tail weights from the loop). scalar/gpsimd
            # queues carry the fp8 weights in parallel.
            qT8 = cpool.tile([128, 2, C], FP8)
            nc.sync.dma_start(out=qT8[:], in_=d_qT8[:])
            ctxT8 = cpool.tile([128, 2, n_pad16], FP8)
            nc.sync.dma_start(out=ctxT8[:], in_=d_ctxT8[:])
            ctxT = cpool.tile([128, 2, n_pad], BF16)
            nc.sync.dma_start(out=ctxT[:], in_=d_ctxT[:])
            qT = cpool.tile([128, 2, C], BF16)
            nc.sync.dma_start(out=qT[:], in_=d_qT[:])
            bh64 = cpool.tile([1, H], BF16)
            nc.sync.dma_start(out=bh64[:], in_=d_bh64[:])
            IndAB = cpool.tile([C + 8, 512], BF16)
            nc.sync.dma_start(out=IndAB[:], in_=d_IndAB[:])
            maskadd = cpool.tile([C, n_pad], F32)
            nc.sync.dma_start(out=maskadd[:], in_=d_maskadd[:])
            ctxa = cpool.tile([n_pad, E], BF16)
            nc.sync.dma_start(out=ctxa[:], in_=d_ctx[:])
            W8A = cpool.tile([128, 2, H], FP8)
            nc.scalar.dma_start(out=W8A[:, :, 0:512], in_=d_W8A[:, :, 0:512])
            nc.scalar.dma_start(out=W8A[:, :, 512:H], in_=d_W8A[:, :, 512:H])
            W8C = cpool.tile([128, 2, 8, 128], FP8)
            nc.scalar.dma_start(out=W8C[:], in_=d_W8C[:])
            Wv8 = cpool.tile([128, 2, 16], FP8)
            nc.scalar.dma_start(out=Wv8[:], in_=d_Wv8[:])
            W8B = cpool.tile([128, 2, H], FP8)
            nc.gpsimd.dma_start(out=W8B[:, :, 0:512], in_=d_W8B[:, :, 0:512])
            nc.gpsimd.dma_start(out=W8B[:, :, 512:H], in_=d_W8B[:, :, 512:H])
            W8D = cpool.tile([128, 2, 8, 128], FP8)
            nc.gpsimd.dma_start(out=W8D[:], in_=d_W8D[:])

            # tail-only tensors; DMAs drip-fed on sync from inside the loop
            WhT = cpool.tile([128, 8, H], BF16)
            bh = cpool.tile([1, H], BF16)
            WlT = cpool.tile([128, 8, E], BF16)
            bl = cpool.tile([128, 2], F32)
            bhT = cpool.tile([128, 8], F32)
            cbT = cpool.tile([128, 3], F32)
            cw = [cpool.tile([128, KS[i], 2, NF], BF16, tag=f"cw{i}",
                             name=f"cw{i}t") for i in range(3)]
            WcT = cpool.tile([128, 3, TYPE_NUM], BF16)
            bc = cpool.tile([TYPE_NUM, 1], F32)
            tail_dmas = [(WhT[:, kc, :], d_WhT[:, kc, :]) for kc in range(8)]
            tail_dmas += [(bh[:], d_bh[:]), (WlT[:], d_WlT[:]),
                          (bl[:], d_bl[:]), (bhT[:], d_bhT[:]),
                          (cbT[:], d_cbT[:])]
            tail_dmas += [(cw[i][:], d_cw[i][:]) for i in range(3)]
            tail_dmas += [(WcT[:], d_WcT[:]), (bc[:], d_bc[:])]

            ones = cpool.tile([1, max(n_pad, C)], BF16)
            nc.vector.memset(ones[:], 1.0)

            # dense broadcast materializations on DVE (doubling copies):
            # qbc[p, ec, t, c] = qT[p, ec, c]; ctxbc[p, ec, t, c] = ctxT[.., t]
            qbc = cpool.tile([128, 2, 8, C], BF16)
            nc.vector.tensor_copy(qbc[:, :, 0, :], qT[:])
            nc.vector.tensor_copy(qbc[:, :, 1, :], qbc[:, :, 0, :])
            nc.vector.tensor_copy(qbc[:, :, 2:4, :], qbc[:, :, 0:2, :])
            nc.vector.tensor_copy(qbc[:, :, 4:8, :], qbc[:, :, 0:4, :])
            ctxbc = cpool.tile([128, 2, n_pad, C], BF16)
            nc.vector.tensor_copy(ctxbc[:, :, :, 0], ctxT[:])
            w = 1
            while w < C:
                nc.vector.tensor_copy(ctxbc[:, :, :, w:2 * w],
                                      ctxbc[:, :, :, 0:w])
                w *= 2

            # PE warm-up burst on junk data: keeps the HAM activity window
            # busy while the first DMAs land so phase 0 runs at full clock.
            junk = cpool.tile([128, 512], BF16)
            nc.vector.memset(junk[:], 0.5)
            psj = ps_sm.tile([128, 512], F32, tag="sm")
            for wi in range(14):
                nc.tensor.matmul(psj[:], junk[:, 0:128], junk[:],
                                 start=(wi == 0), stop=(wi == 13))

            def junk_burst(n):
                # PE-idle bridge: enough matmul activity to stop the HAM
                # clock gate from re-throttling during serial scalar/vector
                # sections. Uses the (then idle) ps_main pool.
                pj = ps_main.tile([128, 2, 512], F32, tag="P")
                for wi in range(n):
                    nc.tensor.matmul(pj[:, 0, :], junk[:, 0:128], junk[:],
                                     start=(wi == 0), stop=(wi == n - 1))

            # ---- phase 0: A = q @ W1.T ; B = ctx @ W2.T + b_h (all x64) ---
            # A rows (c: 0..63) and B rows (t: 64..71, rewritten per rt via
            # SBUF->SBUF DMA) stack into the two alternating AB stationaries.
            AB0 = cpool.tile([C + 8, H], BF16, tag="AB0")
            AB1 = cpool.tile([C + 8, H], BF16, tag="AB1")
            B_T = cpool.tile([n_pad, H], BF16)
            for jn in range(2):
                jsl = slice(jn * 512, (jn + 1) * 512)
                psA = ps_sm.tile([C, 512], F32, tag="sm")
                nc.tensor.matmul(psA[:], qT8[:], W8A[:, :, jsl],
                                 start=True, stop=True, perf_mode=DR)
                nc.scalar.copy(AB0[0:C, jsl], psA[:])
                nc.scalar.copy(AB1[0:C, jsl], psA[:])
                psB = ps_sm.tile([n_pad16, 512], F32, tag="sm")
                nc.tensor.matmul(psB[:], ctxT8[:], W8B[:, :, jsl],
                                 start=True, stop=False, perf_mode=DR)
                nc.tensor.matmul(psB[0:n_pad, :], ones[:, :n_pad],
                                 bh64[:, jsl], start=False, stop=True,
                                 skip_group_check=True)
                nc.scalar.copy(B_T[:, jsl], psB[0:n_pad, :])

            if stage < 2:
                nc.gpsimd.dma_start(out=d_out[:], in_=B_T[0:TYPE_NUM, 0])

            junk_burst(14)  # bridge PE over the phase0 -> rt0 dependency gap

            # ---- phase 1: scores over (c, active t) -----------------------
            scoresT = spool.tile([C, n_pad], F32)
            if stage >= 2:
                ab_tiles = (AB0, AB1)
                tail_fed = 0
                for rt in range(R):
                    AB = ab_tiles[rt % 2]
                    # stationary B rows for this tile -> partitions 64..71
                    nc.sync.dma_start(
                        out=AB[C:C + 8, :],
                        in_=B_T[rt * 8:(rt + 1) * 8, :])
                    if rt >= 2:
                        # drip-feed tail-weight DMAs (3 per rt) on sync
                        for _ in range(3):
                            if tail_fed < len(tail_dmas):
                                o, i_ = tail_dmas[tail_fed]
                                nc.sync.dma_start(out=o, in_=i_)
                                tail_fed += 1
                    ftC = ftpool.tile([128, 2, 8, C], FP8, tag="ftC")
                    ftD = ftpool.tile([128, 2, 8, C], FP8, tag="ftD")
                    sc_t = ftpool.tile([128, 2, 8, C], BF16, tag="sc_t")
                    for ec in range(2):
                        bq = qbc[:, ec]
                        bcx = ctxbc[:, ec, rt * 8:(rt + 1) * 8, :]
                        nc.vector.tensor_sub(sc_t[:, ec], bq, bcx)
                        nc.vector.scalar_tensor_tensor(
                            ftC[:, ec], sc_t[:, ec], -1.0, sc_t[:, ec],
                            op0=ALU.mult, op1=ALU.max)
                        nc.vector.tensor_mul(ftD[:, ec], bq, bcx)
                    # MM order per pair keeps fp8-DR matmuls contiguous (a
                    # bf16->DR mode switch costs ~190ns): 4 DR mains + the
                    # previous pair's DR score matmul, then the 2 bf16
                    # indicator matmuls at the end.
                    S = ps_sm.tile([1, 512], F32, tag="sm")
                    ths = []
                    for jp in range(4):  # pairs of 128-wide hidden chunks
                        P2 = ps_main.tile([128, 2, 512], F32, tag="P")
                        TH2 = thpool.tile([128, 2, 512], FP8, tag="TH")
                        jc0, jc1 = jp * 2, jp * 2 + 1
                        jsl0 = slice(jc0 * 128, (jc0 + 1) * 128)
                        jsl1 = slice(jc1 * 128, (jc1 + 1) * 128)
                        nc.tensor.matmul(P2[:, 0, :], W8C[:, :, jc0, :],
                                         ftC[:], start=True, stop=False,
                                         perf_mode=DR)
                        nc.tensor.matmul(P2[:, 0, :], W8D[:, :, jc0, :],
                                         ftD[:], start=False, stop=False,
                                         perf_mode=DR)
                        if jp > 0:
                            nc.tensor.matmul(S[:], Wv8[:, :, jp - 1:jp],
                                             ths[jp - 1][:], start=(jp == 1),
                                             stop=False, perf_mode=DR,
                                             skip_group_check=True)
                        nc.tensor.matmul(P2[:, 1, :], W8C[:, :, jc1, :],
                                         ftC[:], start=True, stop=False,
                                         perf_mode=DR)
                        nc.tensor.matmul(P2[:, 1, :], W8D[:, :, jc1, :],
                                         ftD[:], start=False, stop=False,
                                         perf_mode=DR)
                        nc.tensor.matmul(P2[:, 0, :], AB[:, jsl0], IndAB[:],
                                         start=False, stop=True,
                                         skip_group_check=True)
                        nc.tensor.matmul(P2[:, 1, :], AB[:, jsl1], IndAB[:],
                                         start=False, stop=True,
                                         skip_group_check=True)
                        nc.scalar.activation(TH2[:], P2[:], AF.Tanh,
                                             scale=1.0 / WS)
                        ths.append(TH2)
                    nc.tensor.matmul(S[:], Wv8[:, :, 3:4], ths[3][:],
                                     start=False, stop=True, perf_mode=DR,
                                     skip_group_check=True)
                    S_sb = thpool.tile([1, 512], F32, tag="S_sb")
                    nc.vector.tensor_scalar_mul(S_sb[:], S[:], 1.0 / WS)
                    # transposed write: d_scr[c, t] (strided, off critical
                    # path) so the final gather is a fast contiguous read
                    nc.scalar.dma_start(
                        out=d_scr[:, rt * 8:(rt + 1) * 8]
                        .rearrange("c t -> t c").unsqueeze(0),
                        in_=S_sb[0:1, :].rearrange("p (t c) -> p t c", c=C))
                while tail_fed < len(tail_dmas):
                    o, i_ = tail_dmas[tail_fed]
                    nc.sync.dma_start(out=o, in_=i_)
                    tail_fed += 1
                nc.scalar.dma_start(out=scoresT[:], in_=d_scr[:])
            if stage == 2:
                nc.sync.dma_start(out=d_out[:], in_=scoresT[0:TYPE_NUM, 0])

            # ---- masked softmax + g = attn @ ctx --------------------------
            if stage >= 3:
                # scores are O(1) so exp() is safe without max-subtraction;
                # masked positions are -1e10 -> exp = 0.
                junk_burst(8)
                nc.vector.tensor_add(scoresT[:], scoresT[:], maskadd[:])
                ex = spool.tile([C, n_pad], F32)
                se = spool.tile([C, 1], F32)
                nc.scalar.activation(ex[:], scoresT[:], AF.Exp,
                                     scale=1.0, accum_out=se[:])
                rse = spool.tile([C, 1], F32)
                nc.vector.reciprocal(rse[:], se[:])
                attn = spool.tile([C, n_pad], BF16)
                nc.vector.tensor_scalar_mul(attn[:], ex[:], rse[:])

                attnT_ps = ps_sm.tile([n_pad, C], BF16, tag="sm")
                nc.tensor.transpose(attnT_ps[:], attn[:], IndAB[0:C, :C])
                attnT = spool.tile([n_pad, C], BF16)
                nc.vector.tensor_copy(attnT[:], attnT_ps[:])
                junk_burst(4)
                # gT[e, c] = (ctx.T @ attn.T)[e, c] -- direct, no transposes
                gT = spool.tile([128, 2, C], BF16)
                for ec in range(2):
                    gT_ps = ps_sm.tile([128, C], F32, tag="sm")
                    nc.tensor.matmul(gT_ps[:],
                                     ctxa[:, ec * 128:(ec + 1) * 128],
                                     attnT[:], start=True, stop=True)
                    nc.scalar.copy(gT[:, ec, :], gT_ps[:])
            if stage == 3:
                nc.gpsimd.dma_start(out=d_out[:], in_=gT[0:TYPE_NUM, 0, 0])

            # ---- phase 2: h2 = tanh([q|g|,|q-g|,q*g] @ Wh.T + bh) ---------
            if stage >= 4:
                junk_burst(4)
                f2C = spool.tile([128, 2, C], BF16)
                f2D = spool.tile([128, 2, C], BF16)
                for ec in range(2):
                    nc.vector.tensor_sub(f2C[:, ec], qT[:, ec, :], gT[:, ec, :])
                    nc.vector.scalar_tensor_tensor(
                        f2C[:, ec], f2C[:, ec], -1.0, f2C[:, ec],
                        op0=ALU.mult, op1=ALU.max)
                    nc.vector.tensor_mul(f2D[:, ec], qT[:, ec, :], gT[:, ec, :])
                h2T = spool.tile([128, 8, C], BF16)
                for jc in range(8):
                    jsl = slice(jc * 128, (jc + 1) * 128)
                    H2 = ps_sm.tile([128, C], F32, tag="sm")
                    for mi, rhs_t in enumerate((qT[:, 0, :], qT[:, 1, :],
                                                gT[:, 0, :], gT[:, 1, :],
                                                f2C[:, 0, :], f2C[:, 1, :],
                                                f2D[:, 0, :], f2D[:, 1, :])):
                        nc.tensor.matmul(H2[:], WhT[:, mi, jsl], rhs_t,
                                         start=(mi == 0), stop=(mi == 7))
                    nc.scalar.activation(h2T[:, jc, :], H2[:], AF.Tanh,
                                         bias=bhT[:, jc:jc + 1], scale=1.0)

                # x.T = W_lin @ h2 : [e, c], e-major for the convs
                xT = spool.tile([128, 2, C], BF16)
                for ec2 in range(2):
                    X = ps_sm.tile([128, C], F32, tag="sm")
                    for jc in range(8):
                        nc.tensor.matmul(
                            X[:], WlT[:, jc, ec2 * 128:(ec2 + 1) * 128],
                            h2T[:, jc, :], start=(jc == 0), stop=(jc == 7))
                    nc.scalar.activation(xT[:, ec2, :], X[:], AF.Identity,
                                         bias=bl[:, ec2:ec2 + 1], scale=1.0)

                # convs + maxpool; conv bias commutes with max over
                # positions, so it folds into the relu bias afterwards
                pooled_raw = spool.tile([NF, 3], F32)
                for i in range(3):
                    ki = KS[i]
                    oi = C - ki + 1
                    Y = ps_sm.tile([NF, oi], F32, tag="sm")
                    nmm = 2 * ki
                    mm = 0
                    for dk in range(ki):
                        for ec2 in range(2):
                            nc.tensor.matmul(Y[:], cw[i][:, dk, ec2, :],
                                             xT[:, ec2, dk:dk + oi],
                                             start=(mm == 0),
                                             stop=(mm == nmm - 1))
                            mm += 1
                    nc.vector.tensor_reduce(pooled_raw[:, i:i + 1], Y[:],
                                            axis=mybir.AxisListType.X,
                                            op=ALU.max)
                pooled = spool.tile([NF, 3], BF16)
                for i in range(3):
                    nc.scalar.activation(pooled[:, i:i + 1],
                                         pooled_raw[:, i:i + 1], AF.Relu,
                                         bias=cbT[:, i:i + 1], scale=1.0)

                # final linear: out = W_cnn @ cnn + b_cnn
                O = ps_sm.tile([TYPE_NUM, 1], F32, tag="sm")
                for i in range(3):
                    nc.tensor.matmul(O[:], WcT[:, i, :], pooled[:, i:i + 1],
                                     start=(i == 0), stop=(i == 2))
                out_sb = spool.tile([TYPE_NUM, 1], F32)
                nc.scalar.activation(out_sb[:], O[:], AF.Identity, bias=bc[:],
                                     scale=1.0)
                nc.sync.dma_start(out=d_out[:], in_=out_sb[:, 0])

    nc.compile()
    nc.m = get_hw_module(nc.m)
    return nc


def _prep_inputs(query, context, mask, W_hidden, b_hidden, W_v, b_v,
                 W_lin, b_lin, conv_w0, conv_b0, conv_w1, conv_b1,
                 conv_w2, conv_b2, W_cnn, b_cnn):
    """Host-side layout prep. Returns (n_pad, per_core_maps)."""
    f32 = np.float32
    mask = np.asarray(mask)
    n_act = mask.sum(1)
    if n_act.min() == 0:
        # degenerate: keep every position, mask on device via maskadd
        idxs = [np.arange(T) for _ in range(B)]
        n_pad = T
        mads = [np.where(mask[b] < 1, NEG, 0.0).astype(f32) for b in range(B)]
    else:
        n_pad = max(8, int(-(-int(n_act.max()) // 8) * 8))
        idxs, mads = [], []
        for b in range(B):
            idx = np.nonzero(mask[b])[0]
            ma = np.full(n_pad, NEG, f32)
            ma[:len(idx)] = 0.0
            idx = np.concatenate([idx, np.zeros(n_pad - len(idx), np.int64)])
            idxs.append(idx)
            mads.append(ma)
    n_pad16 = -(-n_pad // 16) * 16

    bf = bfloat16
    f8 = float8_e4m3
    Wh = np.asarray(W_hidden, f32)
    # W chunk c (of 8) = rows/cols [c*128:(c+1)*128] of the k=1024 dim.
    # WhT[p, kc, j] = Wh[j, kc*128 + p]
    WhT = np.ascontiguousarray(Wh.T).reshape(8, 128, H).transpose(1, 0, 2)
    WhTs = WhT * WS
    # fp8 pair tensors: [ki, pair(2), ...]
    W8A = np.ascontiguousarray(WhTs[:, 0:2, :])            # k chunks 0,1 (q)
    W8B = np.ascontiguousarray(WhTs[:, 2:4, :])            # k chunks 2,3 (ctx)
    W8C = np.ascontiguousarray(
        WhTs[:, 4:6, :].reshape(128, 2, 8, 128))           # |q-ctx|
    W8D = np.ascontiguousarray(
        WhTs[:, 6:8, :].reshape(128, 2, 8, 128))           # q*ctx
    Wv8 = np.zeros((128, 2, 16), f32)
    Wv8[:, :, 0:4] = (np.asarray(W_v, f32)[0] * WS).reshape(4, 2, 128) \
        .transpose(2, 1, 0)

    IndAB = np.concatenate([
        np.tile(np.eye(C, dtype=f32), (1, 8)),
        np.kron(np.eye(8, dtype=f32), np.ones((1, C), f32)),
    ], axis=0)

    query = np.asarray(query, f32)
    qTf = np.ascontiguousarray(query.T.reshape(2, 128, C).transpose(1, 0, 2))
    shared = {
        "qT": qTf.astype(bf),
        "qT8": qTf.astype(f8),
        "W8A": W8A.astype(f8),
        "W8B": W8B.astype(f8),
        "W8C": W8C.astype(f8),
        "W8D": W8D.astype(f8),
        "Wv8": Wv8.astype(f8),
        "IndAB": IndAB.astype(bf),
        "bh64": (np.asarray(b_hidden, f32) * WS).reshape(1, H).astype(bf),
        "WhT": np.ascontiguousarray(WhT).astype(bf),
        "bh": np.asarray(b_hidden, f32).reshape(1, H).astype(bf),
        "WlT": np.ascontiguousarray(
            np.asarray(W_lin, f32).T.reshape(8, 128, E).transpose(1, 0, 2)
        ).astype(bf),
        "bl": np.ascontiguousarray(
            np.asarray(b_lin, f32).reshape(2, 128).T).astype(f32),
        "bhT": np.ascontiguousarray(
            np.asarray(b_hidden, f32).reshape(8, 128).T).astype(f32),
        "cbT": np.stack([np.asarray(x, f32) for x in
                         (conv_b0, conv_b1, conv_b2)], axis=1).astype(f32),
        "WcT": np.ascontiguousarray(
            np.asarray(W_cnn, f32).T.reshape(3, 128, TYPE_NUM)
            .transpose(1, 0, 2)).astype(bf),
        "bc": np.asarray(b_cnn, f32).reshape(TYPE_NUM, 1).astype(f32),
    }
    for i, w in enumerate((conv_w0, conv_w1, conv_w2)):
        w = np.asarray(w, f32)  # [NF, E, ki]
        arr = w.transpose(1, 2, 0).reshape(2, 128, KS[i], NF) \
            .transpose(1, 2, 0, 3)  # [128, ki, 2, NF]
        shared[f"cw{i}"] = np.ascontiguousarray(arr).astype(bf)

    context = np.asarray(context, f32)
    per_core = []
    for b in range(B):
        ctx_act = context[b][idxs[b]]  # [n_pad, E]
        ctx_act = ctx_act * (mads[b] == 0.0)[:, None]  # zero padded rows
        ctxT = np.ascontiguousarray(
            ctx_act.T.reshape(2, 128, n_pad).transpose(1, 0, 2))
        ctxT8 = np.zeros((128, 2, n_pad16), f32)
        ctxT8[:, :, :n_pad] = ctxT
        per_core.append({
            "ctx": np.ascontiguousarray(ctx_act).astype(bf),
            "ctxT": ctxT.astype(bf),
            "ctxT8": ctxT8.astype(f8),
            "maskadd": np.tile(mads[b][None, :], (C, 1)).astype(f32),
            **shared,
        })
    return n_pad, per_core


def kernel(**inputs):
    global LAST_EXEC_NS
    n_pad, per_core = _prep_inputs(**inputs)
    key = (n_pad, os.environ.get("KSTAGE", "99"))
    if key not in _CACHE:
        _CACHE[key] = _build_program(n_pad)
    nc = _CACHE[key]
    res = run_bass_kernel_spmd(nc, per_core, list(range(NUM_CORES)),
                               trace=TRACE)
    LAST_EXEC_NS = res.exec_time_ns
    out = np.stack([res.results[i]["out"] for i in range(NUM_CORES)])
    return out.astype(np.float32)


# revision 33
# speedup vs baseline: 1.2510x; 1.0123x over previous
"""Trainium2 Bass kernel for nn_CNNPredictor (attention scorer + CNN head).

Sharding: data-parallel over batch b (8 batches -> 8 NeuronCores), no
collectives. Each core computes its batch's [TYPE_NUM] output row; host
gathers to [B, TYPE_NUM].

Math (per batch):
  pre[c,t,:] = [q|ctx|, |q-ctx|, q*ctx] @ W_h.T + b_h   (4e = 1024 hidden)
split as
  pre = A[c] + B[t] + W3 @ |q-ctx| + W4 @ (q*ctx)
with A = q @ W1.T, B = ctx @ W2.T + b_h (both tiny, computed in fp8
DoubleRow matmuls). The big K=512 contraction runs in fp8 DoubleRow
(2 matmuls per 128-wide hidden chunk instead of 4 bf16 ones); the A/B
bias is folded in with ONE k=72 bf16 indicator matmul per chunk whose
stationary stacks A rows (c) over B rows (t of this tile). tanh outputs
are fp8 so the W_v reduction also runs DoubleRow (4 matmuls per tile).
W1..W4, b_h, W_v are pre-scaled x64 on the host so fp8 weights stay in
the normal range; the tanh activation un-scales with scale=1/64 and the
score copy un-scales the W_v x64. Only t-positions with mask==1 are
computed (padded to a multiple of 8); masked softmax handles padding.
Softmax + second pass + CNN head stay bf16 (fp8 there breaks the 2e-2
error budget; measured on CPU).
"""

import os
import sys

for _p in ("/opt/trn_rl_repo",):
    if _p not in sys.path:
        sys.path.append(_p)

import numpy as np
from ml_dtypes import bfloat16, float8_e4m3

import concourse.bass as bass
import concourse.bacc as bacc
import concourse.tile as tile
from concourse import mybir
from concourse.bass_utils import run_bass_kernel_spmd
from concourse.bass_interp import get_hw_module

F32 = mybir.dt.float32
BF16 = mybir.dt.bfloat16
FP8 = mybir.dt.float8e4
AF = mybir.ActivationFunctionType
ALU = mybir.AluOpType
DR = mybir.MatmulPerfMode.DoubleRow

B, C, T, E = 8, 64, 128, 256
H = 4 * E  # 1024
NF, TYPE_NUM = 128, 40
KS = (5, 4, 3)
NEG = -1e10
NUM_CORES = 8
WS = 64.0  # host-side scale on W1..W4/bh/Wv so fp8 weights are ~N(0,1)

# module-level knobs for test harness
TRACE = False
LAST_EXEC_NS = None

_CACHE = {}


def _build_program(n_pad):
    """Build the SPMD Bass program for padded active length n_pad (mult of 8)."""
    stage = int(os.environ.get("KSTAGE", "99"))
    R = n_pad // 8  # number of 512-wide r tiles; r = (t, c) t-major
    n_pad16 = -(-n_pad // 16) * 16  # DoubleRow APs need 16B-aligned dim1 step

    nc = bacc.Bacc("TRN2", target_bir_lowering=False, debug=False,
                   num_devices=NUM_CORES)

    # fp8 attention-path tensors (loaded first; small)
    d_qT8 = nc.dram_tensor("qT8", [128, 2, C], FP8, kind="ExternalInput")
    d_ctxT8 = nc.dram_tensor("ctxT8", [128, 2, n_pad16], FP8,
                             kind="ExternalInput")
    d_W8A = nc.dram_tensor("W8A", [128, 2, H], FP8, kind="ExternalInput")
    d_W8B = nc.dram_tensor("W8B", [128, 2, H], FP8, kind="ExternalInput")
    d_W8C = nc.dram_tensor("W8C", [128, 2, 8, 128], FP8, kind="ExternalInput")
    d_W8D = nc.dram_tensor("W8D", [128, 2, 8, 128], FP8, kind="ExternalInput")
    d_Wv8 = nc.dram_tensor("Wv8", [128, 2, 16], FP8, kind="ExternalInput")
    d_IndAB = nc.dram_tensor("IndAB", [C + 8, 512], BF16, kind="ExternalInput")
    d_bh64 = nc.dram_tensor("bh64", [1, H], BF16, kind="ExternalInput")
    d_qT = nc.dram_tensor("qT", [128, 2, C], BF16, kind="ExternalInput")
    d_ctxT = nc.dram_tensor("ctxT", [128, 2, n_pad], BF16, kind="ExternalInput")
    d_maskadd = nc.dram_tensor("maskadd", [n_pad, C], F32, kind="ExternalInput")
    # bf16 tail tensors
    d_ctx = nc.dram_tensor("ctx", [n_pad, E], BF16, kind="ExternalInput")
    d_WhT = nc.dram_tensor("WhT", [128, 8, H], BF16, kind="ExternalInput")
    d_bh = nc.dram_tensor("bh", [1, H], BF16, kind="ExternalInput")
    d_WlT = nc.dram_tensor("WlT", [128, 8, E], BF16, kind="ExternalInput")
    d_bl = nc.dram_tensor("bl", [128, 2], F32, kind="ExternalInput")
    d_cw = [nc.dram_tensor(f"cw{i}", [128, KS[i], 2, NF], BF16,
                           kind="ExternalInput") for i in range(3)]
    d_bhT = nc.dram_tensor("bhT", [128, 8], F32, kind="ExternalInput")
    d_cbT = nc.dram_tensor("cbT", [128, 3], F32, kind="ExternalInput")
    d_WcT = nc.dram_tensor("WcT", [128, 3, TYPE_NUM], BF16, kind="ExternalInput")
    d_bc = nc.dram_tensor("bc", [TYPE_NUM, 1], F32, kind="ExternalInput")
    d_out = nc.dram_tensor("out", [TYPE_NUM], F32, kind="ExternalOutput")

    with tile.TileContext(nc) as tc:
        with (
            tc.tile_pool(name="const", bufs=1) as cpool,
            tc.tile_pool(name="ft", bufs=3) as ftpool,
            tc.tile_pool(name="th", bufs=6) as thpool,
            tc.tile_pool(name="soft", bufs=1) as spool,
            tc.tile_pool(name="ps_main", bufs=3, space="PSUM") as ps_main,
            tc.tile_pool(name="ps_sm", bufs=2, space="PSUM") as ps_sm,
            tc.tile_pool(name="drp", bufs=1, space="DRAM") as drpool,
        ):
            d_scr = drpool.tile([n_pad, C], F32)
            # ---- load constants -------------------------------------------
            # sync queue: small attention-path tensors (plus per-rt AB
            # DMAs and drip-fed tail weights from the loop). scalar/gpsimd
            # queues carry the fp8 weights in parallel.
            qT8 = cpool.tile([128, 2, C], FP8)
            nc.sync.dma_start(out=qT8[:], in_=d_qT8[:])
            ctxT8 = cpool.tile([128, 2, n_pad16], FP8)
            nc.sync.dma_start(out=ctxT8[:], in_=d_ctxT8[:])
            ctxT = cpool.tile([128, 2, n_pad], BF16)
            nc.sync.dma_start(out=ctxT[:], in_=d_ctxT[:])
            qT = cpool.tile([128, 2, C], BF16)
            nc.sync.dma_start(out=qT[:], in_=d_qT[:])
            bh64 = cpool.tile([1, H], BF16)
            nc.sync.dma_start(out=bh64[:], in_=d_bh64[:])
            IndAB = cpool.tile([C + 8, 512], BF16)
            nc.sync.dma_start(out=IndAB[:], in_=d_IndAB[:])
            maskaddT = cpool.tile([n_pad, C], F32)
            nc.sync.dma_start(out=maskaddT[:], in_=d_maskadd[:])
            ctxa = cpool.tile([n_pad, E], BF16)
            nc.sync.dma_start(out=ctxa[:], in_=d_ctx[:])
            W8A = cpool.tile([128, 2, H], FP8)
            nc.scalar.dma_start(out=W8A[:, :, 0:512], in_=d_W8A[:, :, 0:512])
            nc.scalar.dma_start(out=W8A[:, :, 512:H], in_=d_W8A[:, :, 512:H])
            W8C = cpool.tile([128, 2, 8, 128], FP8)
            nc.scalar.dma_start(out=W8C[:], in_=d_W8C[:])
            Wv8 = cpool.tile([128, 2, 16], FP8)
            nc.scalar.dma_start(out=Wv8[:], in_=d_Wv8[:])
            W8B = cpool.tile([128, 2, H], FP8)
            nc.gpsimd.dma_start(out=W8B[:, :, 0:512], in_=d_W8B[:, :, 0:512])
            nc.gpsimd.dma_start(out=W8B[:, :, 512:H], in_=d_W8B[:, :, 512:H])
            W8D = cpool.tile([128, 2, 8, 128], FP8)
            nc.gpsimd.dma_start(out=W8D[:], in_=d_W8D[:])

            # tail-only tensors; DMAs drip-fed on sync from inside the loop
            WhT = cpool.tile([128, 8, H], BF16)
            bh = cpool.tile([1, H], BF16)
            WlT = cpool.tile([128, 8, E], BF16)
            bl = cpool.tile([128, 2], F32)
            bhT = cpool.tile([128, 8], F32)
            cbT = cpool.tile([128, 3], F32)
            cw = [cpool.tile([128, KS[i], 2, NF], BF16, tag=f"cw{i}",
                             name=f"cw{i}t") for i in range(3)]
            WcT = cpool.tile([128, 3, TYPE_NUM], BF16)
            bc = cpool.tile([TYPE_NUM, 1], F32)
            tail_dmas = [(WhT[:, kc, :], d_WhT[:, kc, :]) for kc in range(8)]
            tail_dmas += [(bh[:], d_bh[:]), (WlT[:], d_WlT[:]),
                          (bl[:], d_bl[:]), (bhT[:], d_bhT[:]),
                          (cbT[:], d_cbT[:])]
            tail_dmas += [(cw[i][:], d_cw[i][:]) for i in range(3)]
            tail_dmas += [(WcT[:], d_WcT[:]), (bc[:], d_bc[:])]

            ones = cpool.tile([1, max(n_pad, C)], BF16)
            nc.vector.memset(ones[:], 1.0)
            onesP = cpool.tile([n_pad, 1], BF16)
            nc.vector.memset(onesP[:], 1.0)

            # dense broadcast materializations on DVE (doubling copies):
            # qbc[p, ec, t, c] = qT[p, ec, c]; ctxbc[p, ec, t, c] = ctxT[.., t]
            qbc = cpool.tile([128, 2, 8, C], BF16)
            nc.vector.tensor_copy(qbc[:, :, 0, :], qT[:])
            nc.vector.tensor_copy(qbc[:, :, 1, :], qbc[:, :, 0, :])
            nc.vector.tensor_copy(qbc[:, :, 2:4, :], qbc[:, :, 0:2, :])
            nc.vector.tensor_copy(qbc[:, :, 4:8, :], qbc[:, :, 0:4, :])
            ctxbc = cpool.tile([128, 2, n_pad, C], BF16)
            t0 = min(24, n_pad)
            for tsl in (slice(0, t0), slice(t0, n_pad)):
                if tsl.start >= tsl.stop:
                    continue
                nc.vector.tensor_copy(ctxbc[:, :, tsl, 0], ctxT[:, :, tsl])
                w = 1
                while w < C:
                    nc.vector.tensor_copy(ctxbc[:, :, tsl, w:2 * w],
                                          ctxbc[:, :, tsl, 0:w])
                    w *= 2

            # PE warm-up burst on junk data: keeps the HAM activity window
            # busy while the first DMAs land so phase 0 runs at full clock.
            junk = cpool.tile([128, 512], BF16)
            nc.vector.memset(junk[:], 0.5)
            psj = ps_sm.tile([128, 512], F32, tag="sm")
            for wi in range(14):
                nc.tensor.matmul(psj[:], junk[:, 0:128], junk[:],
                                 start=(wi == 0), stop=(wi == 13))

            def junk_burst(n):
                # PE-idle bridge: enough matmul activity to stop the HAM
                # clock gate from re-throttling during serial scalar/vector
                # sections. Uses the (then idle) ps_main pool.
                pj = ps_main.tile([128, 2, 512], F32, tag="P")
                for wi in range(n):
                    nc.tensor.matmul(pj[:, 0, :], junk[:, 0:128], junk[:],
                                     start=(wi == 0), stop=(wi == n - 1))

            # ---- phase 0: A = q @ W1.T ; B = ctx @ W2.T + b_h (all x64) ---
            # A rows (c: 0..63) and B rows (t: 64..71, rewritten per rt via
            # SBUF->SBUF DMA) stack into the two alternating AB stationaries.
            AB0 = cpool.tile([C + 8, H], BF16, tag="AB0")
            AB1 = cpool.tile([C + 8, H], BF16, tag="AB1")
            B_T = cpool.tile([n_pad, H], BF16)
            for jn in range(2):
                jsl = slice(jn * 512, (jn + 1) * 512)
                psA = ps_sm.tile([C, 512], F32, tag="sm")
                nc.tensor.matmul(psA[:], qT8[:], W8A[:, :, jsl],
                                 start=True, stop=True, perf_mode=DR)
                nc.scalar.copy(AB0[0:C, jsl], psA[:])
                nc.scalar.copy(AB1[0:C, jsl], psA[:])
                psB = ps_sm.tile([n_pad16, 512], F32, tag="sm")
                nc.tensor.matmul(psB[:], ctxT8[:], W8B[:, :, jsl],
                                 start=True, stop=False, perf_mode=DR)
                nc.tensor.matmul(psB[0:n_pad, :], ones[:, :n_pad],
                                 bh64[:, jsl], start=False, stop=True,
                                 skip_group_check=True)
                nc.scalar.copy(B_T[:, jsl], psB[0:n_pad, :])

            if stage < 2:
                nc.gpsimd.dma_start(out=d_out[:], in_=B_T[0:TYPE_NUM, 0])

            junk_burst(14)  # bridge PE over the phase0 -> rt0 dependency gap

            # ---- phase 1: scores over (c, active t) -----------------------
            scoresTT = spool.tile([n_pad, C], F32)
            if stage >= 2:
                ab_tiles = (AB0, AB1)
                tail_fed = 0
                for rt in range(R):
                    AB = ab_tiles[rt % 2]
                    # stationary B rows for this tile -> partitions 64..71
                    nc.sync.dma_start(
                        out=AB[C:C + 8, :],
                        in_=B_T[rt * 8:(rt + 1) * 8, :])
                    if rt >= 2:
                        # drip-feed tail-weight DMAs (3 per rt) on sync
                        for _ in range(3):
                            if tail_fed < len(tail_dmas):
                                o, i_ = tail_dmas[tail_fed]
                                nc.sync.dma_start(out=o, in_=i_)
                                tail_fed += 1
                    ftC = ftpool.tile([128, 2, 8, C], FP8, tag="ftC")
                    ftD = ftpool.tile([128, 2, 8, C], FP8, tag="ftD")
                    sc_t = ftpool.tile([128, 2, 8, C], BF16, tag="sc_t")
                    for ec in range(2):
                        bq = qbc[:, ec]
                        bcx = ctxbc[:, ec, rt * 8:(rt + 1) * 8, :]
                        nc.vector.tensor_sub(sc_t[:, ec], bq, bcx)
                        nc.vector.scalar_tensor_tensor(
                            ftC[:, ec], sc_t[:, ec], -1.0, sc_t[:, ec],
                            op0=ALU.mult, op1=ALU.max)
                        nc.vector.tensor_mul(ftD[:, ec], bq, bcx)
                    # MM order per pair keeps fp8-DR matmuls contiguous (a
                    # bf16->DR mode switch costs ~190ns): 4 DR mains + the
                    # previous pair's DR score matmul, then the 2 bf16
                    # indicator matmuls at the end.
                    S = ps_sm.tile([1, 512], F32, tag="sm")
                    ths = []
                    for jp in range(4):  # pairs of 128-wide hidden chunks
                        P2 = ps_main.tile([128, 2, 512], F32, tag="P")
                        TH2 = thpool.tile([128, 2, 512], FP8, tag="TH")
                        jc0, jc1 = jp * 2, jp * 2 + 1
                        jsl0 = slice(jc0 * 128, (jc0 + 1) * 128)
                        jsl1 = slice(jc1 * 128, (jc1 + 1) * 128)
                        nc.tensor.matmul(P2[:, 0, :], W8C[:, :, jc0, :],
                                         ftC[:], start=True, stop=False,
                                         perf_mode=DR)
                        nc.tensor.matmul(P2[:, 0, :], W8D[:, :, jc0, :],
                                         ftD[:], start=False, stop=False,
                                         perf_mode=DR)
                        if jp > 0:
                            nc.tensor.matmul(S[:], Wv8[:, :, jp - 1:jp],
                                             ths[jp - 1][:], start=(jp == 1),
                                             stop=False, perf_mode=DR,
                                             skip_group_check=True)
                        nc.tensor.matmul(P2[:, 1, :], W8C[:, :, jc1, :],
                                         ftC[:], start=True, stop=False,
                                         perf_mode=DR)
                        nc.tensor.matmul(P2[:, 1, :], W8D[:, :, jc1, :],
                                         ftD[:], start=False, stop=False,
                                         perf_mode=DR)
                        nc.tensor.matmul(P2[:, 0, :], AB[:, jsl0], IndAB[:],
                                         start=False, stop=True,
                                         skip_group_check=True)
                        nc.tensor.matmul(P2[:, 1, :], AB[:, jsl1], IndAB[:],
                                         start=False, stop=True,
                                         skip_group_check=True)
                        nc.scalar.activation(TH2[:], P2[:], AF.Tanh,
                                             scale=1.0 / WS)
                        ths.append(TH2)
                    nc.tensor.matmul(S[:], Wv8[:, :, 3:4], ths[3][:],
                                     start=False, stop=True, perf_mode=DR,
                                     skip_group_check=True)
                    S_sb = thpool.tile([1, 512], F32, tag="S_sb")
                    nc.vector.tensor_scalar_mul(S_sb[:], S[:], 1.0 / WS)
                    nc.gpsimd.dma_start(
                        out=d_scr[rt * 8:(rt + 1) * 8, :].unsqueeze(0),
                        in_=S_sb[0:1, :].rearrange("p (t c) -> p t c", c=C))
                while tail_fed < len(tail_dmas):
                    o, i_ = tail_dmas[tail_fed]
                    nc.sync.dma_start(out=o, in_=i_)
                    tail_fed += 1
                nc.gpsimd.dma_start(out=scoresTT[:], in_=d_scr[:])
            if stage == 2:
                nc.sync.dma_start(out=d_out[:], in_=scoresTT[0:TYPE_NUM, 0])

            # ---- masked softmax + g = attn @ ctx --------------------------
            if stage >= 3:
                # scores are O(1) so exp() is safe without max-subtraction;
                # masked positions are -1e10 -> exp = 0. Everything runs in
                # the native [t, c] layout (t on partitions): the sum over t
                # and the 1/sum broadcast are two tiny PE matmuls, and the
                # gT matmuls consume attnT [t, c] directly -- no transposes.
                junk_burst(8)
                nc.vector.tensor_add(scoresTT[:], scoresTT[:], maskaddT[:])
                exT = spool.tile([n_pad, C], BF16)
                nc.scalar.activation(exT[:], scoresTT[:], AF.Exp, scale=1.0)
                se_ps = ps_sm.tile([1, C], F32, tag="sm")
                nc.tensor.matmul(se_ps[:], onesP[:, 0:1], exT[:],
                                 start=True, stop=True)
                rse = spool.tile([1, C], BF16)
                with nc.allow_low_precision(reason="bf16 1/sum is plenty"):
                    nc.vector.reciprocal(rse[:], se_ps[:])
                rse_ps = ps_sm.tile([n_pad, C], F32, tag="sm")
                nc.tensor.matmul(rse_ps[:], ones[:, :n_pad], rse[:],
                                 start=True, stop=True)
                attnT = spool.tile([n_pad, C], BF16)
                nc.vector.tensor_mul(attnT[:], exT[:], rse_ps[:])
                junk_burst(4)
                # gT[e, c] = (ctx.T @ attn.T)[e, c] -- direct, no transposes
                gT = spool.tile([128, 2, C], BF16)
                for ec in range(2):
                    gT_ps = ps_sm.tile([128, C], F32, tag="sm")
                    nc.tensor.matmul(gT_ps[:],
                                     ctxa[:, ec * 128:(ec + 1) * 128],
                                     attnT[:], start=True, stop=True)
                    nc.scalar.copy(gT[:, ec, :], gT_ps[:])
            if stage == 3:
                nc.gpsimd.dma_start(out=d_out[:], in_=gT[0:TYPE_NUM, 0, 0])

            # ---- phase 2: h2 = tanh([q|g|,|q-g|,q*g] @ Wh.T + bh) ---------
            if stage >= 4:
                junk_burst(4)
                f2C = spool.tile([128, 2, C], BF16)
                f2D = spool.tile([128, 2, C], BF16)
                for ec in range(2):
                    nc.vector.tensor_sub(f2C[:, ec], qT[:, ec, :], gT[:, ec, :])
                    nc.vector.scalar_tensor_tensor(
                        f2C[:, ec], f2C[:, ec], -1.0, f2C[:, ec],
                        op0=ALU.mult, op1=ALU.max)
                    nc.vector.tensor_mul(f2D[:, ec], qT[:, ec, :], gT[:, ec, :])
                h2T = spool.tile([128, 8, C], BF16)
                for jc in range(8):
                    jsl = slice(jc * 128, (jc + 1) * 128)
                    H2 = ps_sm.tile([128, C], F32, tag="sm")
                    for mi, rhs_t in enumerate((qT[:, 0, :], qT[:, 1, :],
                                                gT[:, 0, :], gT[:, 1, :],
                                                f2C[:, 0, :], f2C[:, 1, :],
                                                f2D[:, 0, :], f2D[:, 1, :])):
                        nc.tensor.matmul(H2[:], WhT[:, mi, jsl], rhs_t,
                                         start=(mi == 0), stop=(mi == 7))
                    nc.scalar.activation(h2T[:, jc, :], H2[:], AF.Tanh,
                                         bias=bhT[:, jc:jc + 1], scale=1.0)

                # x.T = W_lin @ h2 : [e, c], e-major for the convs
                xT = spool.tile([128, 2, C], BF16)
                for ec2 in range(2):
                    X = ps_sm.tile([128, C], F32, tag="sm")
                    for jc in range(8):
                        nc.tensor.matmul(
                            X[:], WlT[:, jc, ec2 * 128:(ec2 + 1) * 128],
                            h2T[:, jc, :], start=(jc == 0), stop=(jc == 7))
                    nc.scalar.activation(xT[:, ec2, :], X[:], AF.Identity,
                                         bias=bl[:, ec2:ec2 + 1], scale=1.0)

                # convs + maxpool; conv bias commutes with max over
                # positions, so it folds into the relu bias afterwards
                pooled_raw = spool.tile([NF, 3], F32)
                for i in range(3):
                    ki = KS[i]
                    oi = C - ki + 1
                    Y = ps_sm.tile([NF, oi], F32, tag="sm")
                    nmm = 2 * ki
                    mm = 0
                    for dk in range(ki):
                        for ec2 in range(2):
                            nc.tensor.matmul(Y[:], cw[i][:, dk, ec2, :],
                                             xT[:, ec2, dk:dk + oi],
                                             start=(mm == 0),
                                             stop=(mm == nmm - 1))
                            mm += 1
                    nc.vector.tensor_reduce(pooled_raw[:, i:i + 1], Y[:],
                                            axis=mybir.AxisListType.X,
                                            op=ALU.max)
                pooled = spool.tile([NF, 3], BF16)
                for i in range(3):
                    nc.scalar.activation(pooled[:, i:i + 1],
                                         pooled_raw[:, i:i + 1], AF.Relu,
                                         bias=cbT[:, i:i + 1], scale=1.0)

                # final linear: out = W_cnn @ cnn + b_cnn
                O = ps_sm.tile([TYPE_NUM, 1], F32, tag="sm")
                for i in range(3):
                    nc.tensor.matmul(O[:], WcT[:, i, :], pooled[:, i:i + 1],
                                     start=(i == 0), stop=(i == 2))
                out_sb = spool.tile([TYPE_NUM, 1], F32)
                nc.scalar.activation(out_sb[:], O[:], AF.Identity, bias=bc[:],
                                     scale=1.0)
                nc.sync.dma_start(out=d_out[:], in_=out_sb[:, 0])

    nc.compile()
    nc.m = get_hw_module(nc.m)
    return nc


def _prep_inputs(query, context, mask, W_hidden, b_hidden, W_v, b_v,
                 W_lin, b_lin, conv_w0, conv_b0, conv_w1, conv_b1,
                 conv_w2, conv_b2, W_cnn, b_cnn):
    """Host-side layout prep. Returns (n_pad, per_core_maps)."""
    f32 = np.float32
    mask = np.asarray(mask)
    n_act = mask.sum(1)
    if n_act.min() == 0:
        # degenerate: keep every position, mask on device via maskadd
        idxs = [np.arange(T) for _ in range(B)]
        n_pad = T
        mads = [np.where(mask[b] < 1, NEG, 0.0).astype(f32) for b in range(B)]
    else:
        n_pad = max(8, int(-(-int(n_act.max()) // 8) * 8))
        idxs, mads = [], []
        for b in range(B):
            idx = np.nonzero(mask[b])[0]
            ma = np.full(n_pad, NEG, f32)
            ma[:len(idx)] = 0.0
            idx = np.concatenate([idx, np.zeros(n_pad - len(idx), np.int64)])
            idxs.append(idx)
            mads.append(ma)
    n_pad16 = -(-n_pad // 16) * 16

    bf = bfloat16
    f8 = float8_e4m3
    Wh = np.asarray(W_hidden, f32)
    # W chunk c (of 8) = rows/cols [c*128:(c+1)*128] of the k=1024 dim.
    # WhT[p, kc, j] = Wh[j, kc*128 + p]
    WhT = np.ascontiguousarray(Wh.T).reshape(8, 128, H).transpose(1, 0, 2)
    WhTs = WhT * WS
    # fp8 pair tensors: [ki, pair(2), ...]
    W8A = np.ascontiguousarray(WhTs[:, 0:2, :])            # k chunks 0,1 (q)
    W8B = np.ascontiguousarray(WhTs[:, 2:4, :])            # k chunks 2,3 (ctx)
    W8C = np.ascontiguousarray(
        WhTs[:, 4:6, :].reshape(128, 2, 8, 128))           # |q-ctx|
    W8D = np.ascontiguousarray(
        WhTs[:, 6:8, :].reshape(128, 2, 8, 128))           # q*ctx
    Wv8 = np.zeros((128, 2, 16), f32)
    Wv8[:, :, 0:4] = (np.asarray(W_v, f32)[0] * WS).reshape(4, 2, 128) \
        .transpose(2, 1, 0)

    IndAB = np.concatenate([
        np.tile(np.eye(C, dtype=f32), (1, 8)),
        np.kron(np.eye(8, dtype=f32), np.ones((1, C), f32)),
    ], axis=0)

    query = np.asarray(query, f32)
    qTf = np.ascontiguousarray(query.T.reshape(2, 128, C).transpose(1, 0, 2))
    shared = {
        "qT": qTf.astype(bf),
        "qT8": qTf.astype(f8),
        "W8A": W8A.astype(f8),
        "W8B": W8B.astype(f8),
        "W8C": W8C.astype(f8),
        "W8D": W8D.astype(f8),
        "Wv8": Wv8.astype(f8),
        "IndAB": IndAB.astype(bf),
        "bh64": (np.asarray(b_hidden, f32) * WS).reshape(1, H).astype(bf),
        "WhT": np.ascontiguousarray(WhT).astype(bf),
        "bh": np.asarray(b_hidden, f32).reshape(1, H).astype(bf),
        "WlT": np.ascontiguousarray(
            np.asarray(W_lin, f32).T.reshape(8, 128, E).transpose(1, 0, 2)
        ).astype(bf),
        "bl": np.ascontiguousarray(
            np.asarray(b_lin, f32).reshape(2, 128).T).astype(f32),
        "bhT": np.ascontiguousarray(
            np.asarray(b_hidden, f32).reshape(8, 128).T).astype(f32),
        "cbT": np.stack([np.asarray(x, f32) for x in
                         (conv_b0, conv_b1, conv_b2)], axis=1).astype(f32),
        "WcT": np.ascontiguousarray(
            np.asarray(W_cnn, f32).T.reshape(3, 128, TYPE_NUM)
            .transpose(1, 0, 2)).astype(bf),
        "bc": np.asarray(b_cnn, f32).reshape(TYPE_NUM, 1).astype(f32),
    }
    for i, w in enumerate((conv_w0, conv_w1, conv_w2)):
        w = np.asarray(w, f32)  # [NF, E, ki]
        arr = w.transpose(1, 2, 0).reshape(2, 128, KS[i], NF) \
            .transpose(1, 2, 0, 3)  # [128, ki, 2, NF]
        shared[f"cw{i}"] = np.ascontiguousarray(arr).astype(bf)

    context = np.asarray(context, f32)
    per_core = []
    for b in range(B):
        ctx_act = context[b][idxs[b]]  # [n_pad, E]
        ctx_act = ctx_act * (mads[b] == 0.0)[:, None]  # zero padded rows
        ctxT = np.ascontiguousarray(
            ctx_act.T.reshape(2, 128, n_pad).transpose(1, 0, 2))
        ctxT8 = np.zeros((128, 2, n_pad16), f32)
        ctxT8[:, :, :n_pad] = ctxT
        per_core.append({
            "ctx": np.ascontiguousarray(ctx_act).astype(bf),
            "ctxT": ctxT.astype(bf),
            "ctxT8": ctxT8.astype(f8),
            "maskadd": np.tile(mads[b][:, None], (1, C)).astype(f32),
            **shared,
        })
    return n_pad, per_core


def kernel(**inputs):
    global LAST_EXEC_NS
    n_pad, per_core = _prep_inputs(**inputs)
    key = (n_pad, os.environ.get("KSTAGE", "99"))
    if key not in _CACHE:
        _CACHE[key] = _build_program(n_pad)
    nc = _CACHE[key]
    res = run_bass_kernel_spmd(nc, per_core, list(range(NUM_CORES)),
                               trace=TRACE)
    LAST_EXEC_NS = res.exec_time_ns
    out = np.stack([res.results[i]["out"] for i in range(NUM_CORES)])
    return out.astype(np.float32)


# revision 34
# speedup vs baseline: 1.2830x; 1.0256x over previous
"""Trainium2 Bass kernel for nn_CNNPredictor (attention scorer + CNN head).

Sharding: data-parallel over batch b (8 batches -> 8 NeuronCores), no
collectives. Each core computes its batch's [TYPE_NUM] output row; host
gathers to [B, TYPE_NUM].

Math (per batch):
  pre[c,t,:] = [q|ctx|, |q-ctx|, q*ctx] @ W_h.T + b_h   (4e = 1024 hidden)
split as
  pre = A[c] + B[t] + W3 @ |q-ctx| + W4 @ (q*ctx)
with A = q @ W1.T, B = ctx @ W2.T + b_h (both tiny, computed in fp8
DoubleRow matmuls). The big K=512 contraction runs in fp8 DoubleRow
(2 matmuls per 128-wide hidden chunk instead of 4 bf16 ones); the A/B
bias is folded in with ONE k=72 bf16 indicator matmul per chunk whose
stationary stacks A rows (c) over B rows (t of this tile). tanh outputs
are fp8 so the W_v reduction also runs DoubleRow (4 matmuls per tile).
W1..W4, b_h, W_v are pre-scaled x64 on the host so fp8 weights stay in
the normal range; the tanh activation un-scales with scale=1/64 and the
score copy un-scales the W_v x64. Only t-positions with mask==1 are
computed (padded to a multiple of 8); masked softmax handles padding.
Softmax + second pass + CNN head stay bf16 (fp8 there breaks the 2e-2
error budget; measured on CPU).
"""

import os
import sys

for _p in ("/opt/trn_rl_repo",):
    if _p not in sys.path:
        sys.path.append(_p)

import numpy as np
from ml_dtypes import bfloat16, float8_e4m3

import concourse.bass as bass
import concourse.bacc as bacc
import concourse.tile as tile
from concourse import mybir
from concourse.bass_utils import run_bass_kernel_spmd
from concourse.bass_interp import get_hw_module

F32 = mybir.dt.float32
BF16 = mybir.dt.bfloat16
FP8 = mybir.dt.float8e4
AF = mybir.ActivationFunctionType
ALU = mybir.AluOpType
DR = mybir.MatmulPerfMode.DoubleRow

B, C, T, E = 8, 64, 128, 256
H = 4 * E  # 1024
NF, TYPE_NUM = 128, 40
KS = (5, 4, 3)
NEG = -1e10
NUM_CORES = 8
WS = 64.0  # host-side scale on W1..W4/bh/Wv so fp8 weights are ~N(0,1)

# module-level knobs for test harness
TRACE = False
LAST_EXEC_NS = None

_CACHE = {}


def _build_program(n_pad):
    """Build the SPMD Bass program for padded active length n_pad (mult of 8)."""
    stage = int(os.environ.get("KSTAGE", "99"))
    R = n_pad // 8  # number of 512-wide r tiles; r = (t, c) t-major
    n_pad16 = -(-n_pad // 16) * 16  # DoubleRow APs need 16B-aligned dim1 step

    nc = bacc.Bacc("TRN2", target_bir_lowering=False, debug=False,
                   num_devices=NUM_CORES)

    # fp8 attention-path tensors (loaded first; small)
    d_qT8 = nc.dram_tensor("qT8", [128, 2, C], FP8, kind="ExternalInput")
    d_ctxT8 = nc.dram_tensor("ctxT8", [128, 2, n_pad16], FP8,
                             kind="ExternalInput")
    d_W8A = nc.dram_tensor("W8A", [128, 2, H], FP8, kind="ExternalInput")
    d_W8B = nc.dram_tensor("W8B", [128, 2, H], FP8, kind="ExternalInput")
    d_W8C = nc.dram_tensor("W8C", [128, 2, 8, 128], FP8, kind="ExternalInput")
    d_W8D = nc.dram_tensor("W8D", [128, 2, 8, 128], FP8, kind="ExternalInput")
    d_Wv8 = nc.dram_tensor("Wv8", [128, 2, 16], FP8, kind="ExternalInput")
    d_IndAB = nc.dram_tensor("IndAB", [C + 8, 512], BF16, kind="ExternalInput")
    d_bh64 = nc.dram_tensor("bh64", [1, H], BF16, kind="ExternalInput")
    d_qT = nc.dram_tensor("qT", [128, 2, C], BF16, kind="ExternalInput")
    d_ctxT = nc.dram_tensor("ctxT", [128, 2, n_pad], BF16, kind="ExternalInput")
    d_maskadd = nc.dram_tensor("maskadd", [n_pad, C], F32, kind="ExternalInput")
    # bf16 tail tensors
    d_ctx = nc.dram_tensor("ctx", [n_pad, E], BF16, kind="ExternalInput")
    d_WhT = nc.dram_tensor("WhT", [128, 8, H], BF16, kind="ExternalInput")
    d_bh = nc.dram_tensor("bh", [1, H], BF16, kind="ExternalInput")
    d_WlT = nc.dram_tensor("WlT", [128, 8, E], BF16, kind="ExternalInput")
    d_bl = nc.dram_tensor("bl", [128, 2], F32, kind="ExternalInput")
    d_cw = [nc.dram_tensor(f"cw{i}", [128, KS[i], 2, NF], BF16,
                           kind="ExternalInput") for i in range(3)]
    d_bhT = nc.dram_tensor("bhT", [128, 8], F32, kind="ExternalInput")
    d_cbT = nc.dram_tensor("cbT", [128, 3], F32, kind="ExternalInput")
    d_WcT = nc.dram_tensor("WcT", [128, 3, TYPE_NUM], BF16, kind="ExternalInput")
    d_bc = nc.dram_tensor("bc", [TYPE_NUM, 1], F32, kind="ExternalInput")
    d_out = nc.dram_tensor("out", [TYPE_NUM], F32, kind="ExternalOutput")

    with tile.TileContext(nc) as tc:
        with (
            tc.tile_pool(name="const", bufs=1) as cpool,
            tc.tile_pool(name="ft", bufs=3) as ftpool,
            tc.tile_pool(name="th", bufs=6) as thpool,
            tc.tile_pool(name="soft", bufs=1) as spool,
            tc.tile_pool(name="ps_main", bufs=3, space="PSUM") as ps_main,
            tc.tile_pool(name="ps_sm", bufs=2, space="PSUM") as ps_sm,
            tc.tile_pool(name="drp", bufs=1, space="DRAM") as drpool,
        ):
            # ---- load constants -------------------------------------------
            # sync queue: small attention-path tensors (plus per-rt AB
            # DMAs and drip-fed tail weights from the loop). scalar/gpsimd
            # queues carry the fp8 weights in parallel.
            qT8 = cpool.tile([128, 2, C], FP8)
            nc.sync.dma_start(out=qT8[:], in_=d_qT8[:])
            ctxT8 = cpool.tile([128, 2, n_pad16], FP8)
            nc.sync.dma_start(out=ctxT8[:], in_=d_ctxT8[:])
            ctxT = cpool.tile([128, 2, n_pad], BF16)
            nc.sync.dma_start(out=ctxT[:], in_=d_ctxT[:])
            qT = cpool.tile([128, 2, C], BF16)
            nc.sync.dma_start(out=qT[:], in_=d_qT[:])
            bh64 = cpool.tile([1, H], BF16)
            nc.sync.dma_start(out=bh64[:], in_=d_bh64[:])
            IndAB = cpool.tile([C + 8, 512], BF16)
            nc.sync.dma_start(out=IndAB[:], in_=d_IndAB[:])
            maskaddT = cpool.tile([n_pad, C], F32)
            nc.sync.dma_start(out=maskaddT[:], in_=d_maskadd[:])
            ctxa = cpool.tile([n_pad, E], BF16)
            nc.sync.dma_start(out=ctxa[:], in_=d_ctx[:])
            W8A = cpool.tile([128, 2, H], FP8)
            nc.scalar.dma_start(out=W8A[:, :, 0:512], in_=d_W8A[:, :, 0:512])
            nc.scalar.dma_start(out=W8A[:, :, 512:H], in_=d_W8A[:, :, 512:H])
            W8C = cpool.tile([128, 2, 8, 128], FP8)
            nc.scalar.dma_start(out=W8C[:], in_=d_W8C[:])
            Wv8 = cpool.tile([128, 2, 16], FP8)
            nc.scalar.dma_start(out=Wv8[:], in_=d_Wv8[:])
            W8B = cpool.tile([128, 2, H], FP8)
            nc.gpsimd.dma_start(out=W8B[:, :, 0:512], in_=d_W8B[:, :, 0:512])
            nc.gpsimd.dma_start(out=W8B[:, :, 512:H], in_=d_W8B[:, :, 512:H])
            W8D = cpool.tile([128, 2, 8, 128], FP8)
            nc.gpsimd.dma_start(out=W8D[:], in_=d_W8D[:])

            # tail-only tensors; DMAs drip-fed on sync from inside the loop
            WhT = cpool.tile([128, 8, H], BF16)
            bh = cpool.tile([1, H], BF16)
            WlT = cpool.tile([128, 8, E], BF16)
            bl = cpool.tile([128, 2], F32)
            bhT = cpool.tile([128, 8], F32)
            cbT = cpool.tile([128, 3], F32)
            cw = [cpool.tile([128, KS[i], 2, NF], BF16, tag=f"cw{i}",
                             name=f"cw{i}t") for i in range(3)]
            WcT = cpool.tile([128, 3, TYPE_NUM], BF16)
            bc = cpool.tile([TYPE_NUM, 1], F32)
            tail_dmas = [(WhT[:, kc, :], d_WhT[:, kc, :]) for kc in range(8)]
            tail_dmas += [(bh[:], d_bh[:]), (WlT[:], d_WlT[:]),
                          (bl[:], d_bl[:]), (bhT[:], d_bhT[:]),
                          (cbT[:], d_cbT[:])]
            tail_dmas += [(cw[i][:], d_cw[i][:]) for i in range(3)]
            tail_dmas += [(WcT[:], d_WcT[:]), (bc[:], d_bc[:])]

            ones = cpool.tile([1, max(n_pad, C)], BF16)
            nc.vector.memset(ones[:], 1.0)
            onesP = cpool.tile([n_pad, 1], BF16)
            nc.vector.memset(onesP[:], 1.0)

            # dense broadcast materializations on DVE (doubling copies):
            # qbc[p, ec, t, c] = qT[p, ec, c]; ctxbc[p, ec, t, c] = ctxT[.., t]
            qbc = cpool.tile([128, 2, 8, C], BF16)
            nc.vector.tensor_copy(qbc[:, :, 0, :], qT[:])
            nc.vector.tensor_copy(qbc[:, :, 1, :], qbc[:, :, 0, :])
            nc.vector.tensor_copy(qbc[:, :, 2:4, :], qbc[:, :, 0:2, :])
            nc.vector.tensor_copy(qbc[:, :, 4:8, :], qbc[:, :, 0:4, :])
            ctxbc = cpool.tile([128, 2, n_pad, C], BF16)
            cuts = sorted({0, min(8, n_pad), min(32, n_pad), n_pad})
            for tsl in [slice(a, b) for a, b in zip(cuts, cuts[1:])]:
                if tsl.start >= tsl.stop:
                    continue
                nc.vector.tensor_copy(ctxbc[:, :, tsl, 0], ctxT[:, :, tsl])
                w = 1
                while w < C:
                    nc.vector.tensor_copy(ctxbc[:, :, tsl, w:2 * w],
                                          ctxbc[:, :, tsl, 0:w])
                    w *= 2

            # PE warm-up burst on junk data: keeps the HAM activity window
            # busy while the first DMAs land so phase 0 runs at full clock.
            junk = cpool.tile([128, 512], BF16)
            nc.vector.memset(junk[:], 0.5)
            psj = ps_sm.tile([128, 512], F32, tag="sm")
            for wi in range(14):
                nc.tensor.matmul(psj[:], junk[:, 0:128], junk[:],
                                 start=(wi == 0), stop=(wi == 13))

            def junk_burst(n):
                # PE-idle bridge: enough matmul activity to stop the HAM
                # clock gate from re-throttling during serial scalar/vector
                # sections. Uses the (then idle) ps_main pool.
                pj = ps_main.tile([128, 2, 512], F32, tag="P")
                for wi in range(n):
                    nc.tensor.matmul(pj[:, 0, :], junk[:, 0:128], junk[:],
                                     start=(wi == 0), stop=(wi == n - 1))

            # ---- phase 0: A = q @ W1.T ; B = ctx @ W2.T + b_h (all x64) ---
            # A rows (c: 0..63) and B rows (t: 64..71, rewritten per rt via
            # SBUF->SBUF DMA) stack into the two alternating AB stationaries.
            AB0 = cpool.tile([C + 8, H], BF16, tag="AB0")
            AB1 = cpool.tile([C + 8, H], BF16, tag="AB1")
            B_T = cpool.tile([n_pad, H], BF16)
            for jn in range(2):
                jsl = slice(jn * 512, (jn + 1) * 512)
                psA = ps_sm.tile([C, 512], F32, tag="sm")
                nc.tensor.matmul(psA[:], qT8[:], W8A[:, :, jsl],
                                 start=True, stop=True, perf_mode=DR)
                nc.scalar.copy(AB0[0:C, jsl], psA[:])
                nc.scalar.copy(AB1[0:C, jsl], psA[:])
            for jn in range(2):
                jsl = slice(jn * 512, (jn + 1) * 512)
                psB = ps_sm.tile([n_pad16, 512], F32, tag="sm")
                nc.tensor.matmul(psB[:], ctxT8[:], W8B[:, :, jsl],
                                 start=True, stop=False, perf_mode=DR)
                nc.tensor.matmul(psB[0:n_pad, :], ones[:, :n_pad],
                                 bh64[:, jsl], start=False, stop=True,
                                 skip_group_check=True)
                nc.scalar.copy(B_T[:, jsl], psB[0:n_pad, :])

            if stage < 2:
                nc.gpsimd.dma_start(out=d_out[:], in_=B_T[0:TYPE_NUM, 0])

            junk_burst(14)  # bridge PE over the phase0 -> rt0 dependency gap

            # ---- phase 1: scores over (c, active t) -----------------------
            scoresTT = spool.tile([n_pad, C], F32)
            if stage >= 2:
                ab_tiles = (AB0, AB1)
                tail_fed = 0
                for rt in range(R):
                    AB = ab_tiles[rt % 2]
                    # stationary B rows for this tile -> partitions 64..71
                    nc.sync.dma_start(
                        out=AB[C:C + 8, :],
                        in_=B_T[rt * 8:(rt + 1) * 8, :])
                    if rt >= 2:
                        # drip-feed tail-weight DMAs (3 per rt) on sync
                        for _ in range(3):
                            if tail_fed < len(tail_dmas):
                                o, i_ = tail_dmas[tail_fed]
                                nc.sync.dma_start(out=o, in_=i_)
                                tail_fed += 1
                    ftC = ftpool.tile([128, 2, 8, C], FP8, tag="ftC")
                    ftD = ftpool.tile([128, 2, 8, C], FP8, tag="ftD")
                    sc_t = ftpool.tile([128, 2, 8, C], BF16, tag="sc_t")
                    for ec in range(2):
                        bq = qbc[:, ec]
                        bcx = ctxbc[:, ec, rt * 8:(rt + 1) * 8, :]
                        nc.vector.tensor_sub(sc_t[:, ec], bq, bcx)
                        nc.vector.scalar_tensor_tensor(
                            ftC[:, ec], sc_t[:, ec], -1.0, sc_t[:, ec],
                            op0=ALU.mult, op1=ALU.max)
                        nc.vector.tensor_mul(ftD[:, ec], bq, bcx)
                    # MM order per pair keeps fp8-DR matmuls contiguous (a
                    # bf16->DR mode switch costs ~190ns): 4 DR mains + the
                    # previous pair's DR score matmul, then the 2 bf16
                    # indicator matmuls at the end.
                    S = ps_sm.tile([1, 512], F32, tag="sm")
                    ths = []
                    for jp in range(4):  # pairs of 128-wide hidden chunks
                        P2 = ps_main.tile([128, 2, 512], F32, tag="P")
                        TH2 = thpool.tile([128, 2, 512], FP8, tag="TH")
                        jc0, jc1 = jp * 2, jp * 2 + 1
                        jsl0 = slice(jc0 * 128, (jc0 + 1) * 128)
                        jsl1 = slice(jc1 * 128, (jc1 + 1) * 128)
                        nc.tensor.matmul(P2[:, 0, :], W8C[:, :, jc0, :],
                                         ftC[:], start=True, stop=False,
                                         perf_mode=DR)
                        nc.tensor.matmul(P2[:, 0, :], W8D[:, :, jc0, :],
                                         ftD[:], start=False, stop=False,
                                         perf_mode=DR)
                        if jp > 0:
                            nc.tensor.matmul(S[:], Wv8[:, :, jp - 1:jp],
                                             ths[jp - 1][:], start=(jp == 1),
                                             stop=False, perf_mode=DR,
                                             skip_group_check=True)
                        nc.tensor.matmul(P2[:, 1, :], W8C[:, :, jc1, :],
                                         ftC[:], start=True, stop=False,
                                         perf_mode=DR)
                        nc.tensor.matmul(P2[:, 1, :], W8D[:, :, jc1, :],
                                         ftD[:], start=False, stop=False,
                                         perf_mode=DR)
                        nc.tensor.matmul(P2[:, 0, :], AB[:, jsl0], IndAB[:],
                                         start=False, stop=True,
                                         skip_group_check=True)
                        nc.tensor.matmul(P2[:, 1, :], AB[:, jsl1], IndAB[:],
                                         start=False, stop=True,
                                         skip_group_check=True)
                        nc.scalar.activation(TH2[:], P2[:], AF.Tanh,
                                             scale=1.0 / WS)
                        ths.append(TH2)
                    nc.tensor.matmul(S[:], Wv8[:, :, 3:4], ths[3][:],
                                     start=False, stop=True, perf_mode=DR,
                                     skip_group_check=True)
                    S_sb = thpool.tile([1, 512], F32, tag="S_sb")
                    nc.vector.tensor_scalar_mul(S_sb[:], S[:], 1.0 / WS)
                    # partition-scatter 1x512 -> 8 t-rows of scoresTT
                    nc.gpsimd.dma_start(
                        out=scoresTT[rt * 8:(rt + 1) * 8, :],
                        in_=S_sb[0:1, :].rearrange("p (t c) -> p t c", c=C))
                while tail_fed < len(tail_dmas):
                    o, i_ = tail_dmas[tail_fed]
                    nc.sync.dma_start(out=o, in_=i_)
                    tail_fed += 1
            if stage == 2:
                nc.sync.dma_start(out=d_out[:], in_=scoresTT[0:TYPE_NUM, 0])

            # ---- masked softmax + g = attn @ ctx --------------------------
            if stage >= 3:
                # scores are O(1) so exp() is safe without max-subtraction;
                # masked positions are -1e10 -> exp = 0. Everything runs in
                # the native [t, c] layout (t on partitions): the sum over t
                # and the 1/sum broadcast are two tiny PE matmuls, and the
                # gT matmuls consume attnT [t, c] directly -- no transposes.
                junk_burst(8)
                nc.vector.tensor_add(scoresTT[:], scoresTT[:], maskaddT[:])
                exT = spool.tile([n_pad, C], BF16)
                nc.scalar.activation(exT[:], scoresTT[:], AF.Exp, scale=1.0)
                se_ps = ps_sm.tile([1, C], F32, tag="sm")
                nc.tensor.matmul(se_ps[:], onesP[:, 0:1], exT[:],
                                 start=True, stop=True)
                rse = spool.tile([1, C], BF16)
                with nc.allow_low_precision(reason="bf16 1/sum is plenty"):
                    nc.vector.reciprocal(rse[:], se_ps[:])
                rse_ps = ps_sm.tile([n_pad, C], F32, tag="sm")
                nc.tensor.matmul(rse_ps[:], ones[:, :n_pad], rse[:],
                                 start=True, stop=True)
                attnT = spool.tile([n_pad, C], BF16)
                nc.vector.tensor_mul(attnT[:], exT[:], rse_ps[:])
                junk_burst(4)
                # gT[e, c] = (ctx.T @ attn.T)[e, c] -- direct, no transposes
                gT = spool.tile([128, 2, C], BF16)
                for ec in range(2):
                    gT_ps = ps_sm.tile([128, C], F32, tag="sm")
                    nc.tensor.matmul(gT_ps[:],
                                     ctxa[:, ec * 128:(ec + 1) * 128],
                                     attnT[:], start=True, stop=True)
                    nc.scalar.copy(gT[:, ec, :], gT_ps[:])
            if stage == 3:
                nc.gpsimd.dma_start(out=d_out[:], in_=gT[0:TYPE_NUM, 0, 0])

            # ---- phase 2: h2 = tanh([q|g|,|q-g|,q*g] @ Wh.T + bh) ---------
            if stage >= 4:
                junk_burst(4)
                f2C = spool.tile([128, 2, C], BF16)
                f2D = spool.tile([128, 2, C], BF16)
                for ec in range(2):
                    nc.vector.tensor_sub(f2C[:, ec], qT[:, ec, :], gT[:, ec, :])
                    nc.vector.scalar_tensor_tensor(
                        f2C[:, ec], f2C[:, ec], -1.0, f2C[:, ec],
                        op0=ALU.mult, op1=ALU.max)
                    nc.vector.tensor_mul(f2D[:, ec], qT[:, ec, :], gT[:, ec, :])
                h2T = spool.tile([128, 8, C], BF16)
                for jc in range(8):
                    jsl = slice(jc * 128, (jc + 1) * 128)
                    H2 = ps_sm.tile([128, C], F32, tag="sm")
                    for mi, rhs_t in enumerate((qT[:, 0, :], qT[:, 1, :],
                                                gT[:, 0, :], gT[:, 1, :],
                                                f2C[:, 0, :], f2C[:, 1, :],
                                                f2D[:, 0, :], f2D[:, 1, :])):
                        nc.tensor.matmul(H2[:], WhT[:, mi, jsl], rhs_t,
                                         start=(mi == 0), stop=(mi == 7))
                    nc.scalar.activation(h2T[:, jc, :], H2[:], AF.Tanh,
                                         bias=bhT[:, jc:jc + 1], scale=1.0)

                # x.T = W_lin @ h2 : [e, c], e-major for the convs
                xT = spool.tile([128, 2, C], BF16)
                for ec2 in range(2):
                    X = ps_sm.tile([128, C], F32, tag="sm")
                    for jc in range(8):
                        nc.tensor.matmul(
                            X[:], WlT[:, jc, ec2 * 128:(ec2 + 1) * 128],
                            h2T[:, jc, :], start=(jc == 0), stop=(jc == 7))
                    nc.scalar.activation(xT[:, ec2, :], X[:], AF.Identity,
                                         bias=bl[:, ec2:ec2 + 1], scale=1.0)

                # convs + maxpool; conv bias commutes with max over
                # positions, so it folds into the relu bias afterwards
                pooled_raw = spool.tile([NF, 3], F32)
                for i in range(3):
                    ki = KS[i]
                    oi = C - ki + 1
                    Y = ps_sm.tile([NF, oi], F32, tag="sm")
                    nmm = 2 * ki
                    mm = 0
                    for dk in range(ki):
                        for ec2 in range(2):
                            nc.tensor.matmul(Y[:], cw[i][:, dk, ec2, :],
                                             xT[:, ec2, dk:dk + oi],
                                             start=(mm == 0),
                                             stop=(mm == nmm - 1))
                            mm += 1
                    nc.vector.tensor_reduce(pooled_raw[:, i:i + 1], Y[:],
                                            axis=mybir.AxisListType.X,
                                            op=ALU.max)
                pooled = spool.tile([NF, 3], BF16)
                for i in range(3):
                    nc.scalar.activation(pooled[:, i:i + 1],
                                         pooled_raw[:, i:i + 1], AF.Relu,
                                         bias=cbT[:, i:i + 1], scale=1.0)

                # final linear: out = W_cnn @ cnn + b_cnn
                O = ps_sm.tile([TYPE_NUM, 1], F32, tag="sm")
                for i in range(3):
                    nc.tensor.matmul(O[:], WcT[:, i, :], pooled[:, i:i + 1],
                                     start=(i == 0), stop=(i == 2))
                out_sb = spool.tile([TYPE_NUM, 1], F32)
                nc.scalar.activation(out_sb[:], O[:], AF.Identity, bias=bc[:],
                                     scale=1.0)
                nc.sync.dma_start(out=d_out[:], in_=out_sb[:, 0])

    nc.compile()
    nc.m = get_hw_module(nc.m)
    return nc


def _prep_inputs(query, context, mask, W_hidden, b_hidden, W_v, b_v,
                 W_lin, b_lin, conv_w0, conv_b0, conv_w1, conv_b1,
                 conv_w2, conv_b2, W_cnn, b_cnn):
    """Host-side layout prep. Returns (n_pad, per_core_maps)."""
    f32 = np.float32
    mask = np.asarray(mask)
    n_act = mask.sum(1)
    if n_act.min() == 0:
        # degenerate: keep every position, mask on device via maskadd
        idxs = [np.arange(T) for _ in range(B)]
        n_pad = T
        mads = [np.where(mask[b] < 1, NEG, 0.0).astype(f32) for b in range(B)]
    else:
        n_pad = max(8, int(-(-int(n_act.max()) // 8) * 8))
        idxs, mads = [], []
        for b in range(B):
            idx = np.nonzero(mask[b])[0]
            ma = np.full(n_pad, NEG, f32)
            ma[:len(idx)] = 0.0
            idx = np.concatenate([idx, np.zeros(n_pad - len(idx), np.int64)])
            idxs.append(idx)
            mads.append(ma)
    n_pad16 = -(-n_pad // 16) * 16

    bf = bfloat16
    f8 = float8_e4m3
    Wh = np.asarray(W_hidden, f32)
    # W chunk c (of 8) = rows/cols [c*128:(c+1)*128] of the k=1024 dim.
    # WhT[p, kc, j] = Wh[j, kc*128 + p]
    WhT = np.ascontiguousarray(Wh.T).reshape(8, 128, H).transpose(1, 0, 2)
    WhTs = WhT * WS
    # fp8 pair tensors: [ki, pair(2), ...]
    W8A = np.ascontiguousarray(WhTs[:, 0:2, :])            # k chunks 0,1 (q)
    W8B = np.ascontiguousarray(WhTs[:, 2:4, :])            # k chunks 2,3 (ctx)
    W8C = np.ascontiguousarray(
        WhTs[:, 4:6, :].reshape(128, 2, 8, 128))           # |q-ctx|
    W8D = np.ascontiguousarray(
        WhTs[:, 6:8, :].reshape(128, 2, 8, 128))           # q*ctx
    Wv8 = np.zeros((128, 2, 16), f32)
    Wv8[:, :, 0:4] = (np.asarray(W_v, f32)[0] * WS).reshape(4, 2, 128) \
        .transpose(2, 1, 0)

    IndAB = np.concatenate([
        np.tile(np.eye(C, dtype=f32), (1, 8)),
        np.kron(np.eye(8, dtype=f32), np.ones((1, C), f32)),
    ], axis=0)

    query = np.asarray(query, f32)
    qTf = np.ascontiguousarray(query.T.reshape(2, 128, C).transpose(1, 0, 2))
    shared = {
        "qT": qTf.astype(bf),
        "qT8": qTf.astype(f8),
        "W8A": W8A.astype(f8),
        "W8B": W8B.astype(f8),
        "W8C": W8C.astype(f8),
        "W8D": W8D.astype(f8),
        "Wv8": Wv8.astype(f8),
        "IndAB": IndAB.astype(bf),
        "bh64": (np.asarray(b_hidden, f32) * WS).reshape(1, H).astype(bf),
        "WhT": np.ascontiguousarray(WhT).astype(bf),
        "bh": np.asarray(b_hidden, f32).reshape(1, H).astype(bf),
        "WlT": np.ascontiguousarray(
            np.asarray(W_lin, f32).T.reshape(8, 128, E).transpose(1, 0, 2)
        ).astype(bf),
        "bl": np.ascontiguousarray(
            np.asarray(b_lin, f32).reshape(2, 128).T).astype(f32),
        "bhT": np.ascontiguousarray(
            np.asarray(b_hidden, f32).reshape(8, 128).T).astype(f32),
        "cbT": np.stack([np.asarray(x, f32) for x in
                         (conv_b0, conv_b1, conv_b2)], axis=1).astype(f32),
        "WcT": np.ascontiguousarray(
            np.asarray(W_cnn, f32).T.reshape(3, 128, TYPE_NUM)
            .transpose(1, 0, 2)).astype(bf),
        "bc": np.asarray(b_cnn, f32).reshape(TYPE_NUM, 1).astype(f32),
    }
    for i, w in enumerate((conv_w0, conv_w1, conv_w2)):
        w = np.asarray(w, f32)  # [NF, E, ki]
        arr = w.transpose(1, 2, 0).reshape(2, 128, KS[i], NF) \
            .transpose(1, 2, 0, 3)  # [128, ki, 2, NF]
        shared[f"cw{i}"] = np.ascontiguousarray(arr).astype(bf)

    context = np.asarray(context, f32)
    per_core = []
    for b in range(B):
        ctx_act = context[b][idxs[b]]  # [n_pad, E]
        ctx_act = ctx_act * (mads[b] == 0.0)[:, None]  # zero padded rows
        ctxT = np.ascontiguousarray(
            ctx_act.T.reshape(2, 128, n_pad).transpose(1, 0, 2))
        ctxT8 = np.zeros((128, 2, n_pad16), f32)
        ctxT8[:, :, :n_pad] = ctxT
        per_core.append({
            "ctx": np.ascontiguousarray(ctx_act).astype(bf),
            "ctxT": ctxT.astype(bf),
            "ctxT8": ctxT8.astype(f8),
            "maskadd": np.tile(mads[b][:, None], (1, C)).astype(f32),
            **shared,
        })
    return n_pad, per_core


def kernel(**inputs):
    global LAST_EXEC_NS
    n_pad, per_core = _prep_inputs(**inputs)
    key = (n_pad, os.environ.get("KSTAGE", "99"))
    if key not in _CACHE:
        _CACHE[key] = _build_program(n_pad)
    nc = _CACHE[key]
    res = run_bass_kernel_spmd(nc, per_core, list(range(NUM_CORES)),
                               trace=TRACE)
    LAST_EXEC_NS = res.exec_time_ns
    out = np.stack([res.results[i]["out"] for i in range(NUM_CORES)])
    return out.astype(np.float32)


# revision 35
# speedup vs baseline: 1.2879x; 1.0038x over previous
"""Trainium2 Bass kernel for nn_CNNPredictor (attention scorer + CNN head).

Sharding: data-parallel over batch b (8 batches -> 8 NeuronCores), no
collectives. Each core computes its batch's [TYPE_NUM] output row; host
gathers to [B, TYPE_NUM].

Math (per batch):
  pre[c,t,:] = [q|ctx|, |q-ctx|, q*ctx] @ W_h.T + b_h   (4e = 1024 hidden)
split as
  pre = A[c] + B[t] + W3 @ |q-ctx| + W4 @ (q*ctx)
with A = q @ W1.T, B = ctx @ W2.T + b_h (both tiny, computed in fp8
DoubleRow matmuls). The big K=512 contraction runs in fp8 DoubleRow
(2 matmuls per 128-wide hidden chunk instead of 4 bf16 ones); the A/B
bias is folded in with ONE k=72 bf16 indicator matmul per chunk whose
stationary stacks A rows (c) over B rows (t of this tile). tanh outputs
are fp8 so the W_v reduction also runs DoubleRow (4 matmuls per tile).
W1..W4, b_h, W_v are pre-scaled x64 on the host so fp8 weights stay in
the normal range; the tanh activation un-scales with scale=1/64 and the
score copy un-scales the W_v x64. Only t-positions with mask==1 are
computed (padded to a multiple of 8); masked softmax handles padding.
Softmax + second pass + CNN head stay bf16 (fp8 there breaks the 2e-2
error budget; measured on CPU).
"""

import os
import sys

for _p in ("/opt/trn_rl_repo",):
    if _p not in sys.path:
        sys.path.append(_p)

import numpy as np
from ml_dtypes import bfloat16, float8_e4m3

import concourse.bass as bass
import concourse.bacc as bacc
import concourse.tile as tile
from concourse import mybir
from concourse.bass_utils import run_bass_kernel_spmd
from concourse.bass_interp import get_hw_module

F32 = mybir.dt.float32
BF16 = mybir.dt.bfloat16
FP8 = mybir.dt.float8e4
AF = mybir.ActivationFunctionType
ALU = mybir.AluOpType
DR = mybir.MatmulPerfMode.DoubleRow

B, C, T, E = 8, 64, 128, 256
H = 4 * E  # 1024
NF, TYPE_NUM = 128, 40
KS = (5, 4, 3)
NEG = -1e10
NUM_CORES = 8
WS = 64.0  # host-side scale on W1..W4/bh/Wv so fp8 weights are ~N(0,1)

# module-level knobs for test harness
TRACE = False
LAST_EXEC_NS = None

_CACHE = {}


def _build_program(n_pad):
    """Build the SPMD Bass program for padded active length n_pad (mult of 8)."""
    stage = int(os.environ.get("KSTAGE", "99"))
    R = n_pad // 8  # number of 512-wide r tiles; r = (t, c) t-major
    n_pad16 = -(-n_pad // 16) * 16  # DoubleRow APs need 16B-aligned dim1 step

    nc = bacc.Bacc("TRN2", target_bir_lowering=False, debug=False,
                   num_devices=NUM_CORES)

    # fp8 attention-path tensors (loaded first; small)
    d_qT8 = nc.dram_tensor("qT8", [128, 2, C], FP8, kind="ExternalInput")
    d_ctxT8 = nc.dram_tensor("ctxT8", [128, 2, n_pad16], FP8,
                             kind="ExternalInput")
    d_W8A = nc.dram_tensor("W8A", [128, 2, H], FP8, kind="ExternalInput")
    d_W8B = nc.dram_tensor("W8B", [128, 2, H], FP8, kind="ExternalInput")
    d_W8C = nc.dram_tensor("W8C", [128, 2, 8, 128], FP8, kind="ExternalInput")
    d_W8D = nc.dram_tensor("W8D", [128, 2, 8, 128], FP8, kind="ExternalInput")
    d_Wv8 = nc.dram_tensor("Wv8", [128, 2, 16], FP8, kind="ExternalInput")
    d_IndAB = nc.dram_tensor("IndAB", [C + 8, 512], BF16, kind="ExternalInput")
    d_bh64 = nc.dram_tensor("bh64", [1, H], BF16, kind="ExternalInput")
    d_qT = nc.dram_tensor("qT", [128, 2, C], BF16, kind="ExternalInput")
    d_ctxT = nc.dram_tensor("ctxT", [128, 2, n_pad], BF16, kind="ExternalInput")
    d_maskadd = nc.dram_tensor("maskadd", [n_pad, C], F32, kind="ExternalInput")
    # bf16 tail tensors
    d_ctx = nc.dram_tensor("ctx", [n_pad, E], BF16, kind="ExternalInput")
    d_WhT = nc.dram_tensor("WhT", [128, 8, H], BF16, kind="ExternalInput")
    d_bh = nc.dram_tensor("bh", [1, H], BF16, kind="ExternalInput")
    d_WlT = nc.dram_tensor("WlT", [128, 8, E], BF16, kind="ExternalInput")
    d_bl = nc.dram_tensor("bl", [128, 2], F32, kind="ExternalInput")
    d_cw = [nc.dram_tensor(f"cw{i}", [128, KS[i], 2, NF], BF16,
                           kind="ExternalInput") for i in range(3)]
    d_bhT = nc.dram_tensor("bhT", [128, 8], F32, kind="ExternalInput")
    d_cbT = nc.dram_tensor("cbT", [128, 3], F32, kind="ExternalInput")
    d_WcT = nc.dram_tensor("WcT", [128, 3, TYPE_NUM], BF16, kind="ExternalInput")
    d_bc = nc.dram_tensor("bc", [TYPE_NUM, 1], F32, kind="ExternalInput")
    d_out = nc.dram_tensor("out", [TYPE_NUM], F32, kind="ExternalOutput")

    with tile.TileContext(nc) as tc:
        with (
            tc.tile_pool(name="const", bufs=1) as cpool,
            tc.tile_pool(name="ft", bufs=3) as ftpool,
            tc.tile_pool(name="th", bufs=6) as thpool,
            tc.tile_pool(name="soft", bufs=1) as spool,
            tc.tile_pool(name="ps_main", bufs=3, space="PSUM") as ps_main,
            tc.tile_pool(name="ps_sm", bufs=2, space="PSUM") as ps_sm,
            tc.tile_pool(name="drp", bufs=1, space="DRAM") as drpool,
        ):
            # ---- load constants -------------------------------------------
            # sync queue: small attention-path tensors (plus per-rt AB
            # DMAs and drip-fed tail weights from the loop). scalar/gpsimd
            # queues carry the fp8 weights in parallel.
            qT8 = cpool.tile([128, 2, C], FP8)
            nc.sync.dma_start(out=qT8[:], in_=d_qT8[:])
            ctxT8 = cpool.tile([128, 2, n_pad16], FP8)
            nc.sync.dma_start(out=ctxT8[:], in_=d_ctxT8[:])
            ctxT = cpool.tile([128, 2, n_pad], BF16)
            nc.sync.dma_start(out=ctxT[:], in_=d_ctxT[:])
            qT = cpool.tile([128, 2, C], BF16)
            nc.sync.dma_start(out=qT[:], in_=d_qT[:])
            bh64 = cpool.tile([1, H], BF16)
            nc.sync.dma_start(out=bh64[:], in_=d_bh64[:])
            IndAB = cpool.tile([C + 8, 512], BF16)
            nc.sync.dma_start(out=IndAB[:], in_=d_IndAB[:])
            maskaddT = cpool.tile([n_pad, C], F32)
            nc.sync.dma_start(out=maskaddT[:], in_=d_maskadd[:])
            ctxa = cpool.tile([n_pad, E], BF16)
            nc.sync.dma_start(out=ctxa[:], in_=d_ctx[:])
            W8A = cpool.tile([128, 2, H], FP8)
            nc.scalar.dma_start(out=W8A[:, :, 0:512], in_=d_W8A[:, :, 0:512])
            nc.scalar.dma_start(out=W8A[:, :, 512:H], in_=d_W8A[:, :, 512:H])
            W8C = cpool.tile([128, 2, 8, 128], FP8)
            nc.scalar.dma_start(out=W8C[:], in_=d_W8C[:])
            Wv8 = cpool.tile([128, 2, 16], FP8)
            nc.scalar.dma_start(out=Wv8[:], in_=d_Wv8[:])
            W8B = cpool.tile([128, 2, H], FP8)
            nc.gpsimd.dma_start(out=W8B[:, :, 0:512], in_=d_W8B[:, :, 0:512])
            nc.gpsimd.dma_start(out=W8B[:, :, 512:H], in_=d_W8B[:, :, 512:H])
            W8D = cpool.tile([128, 2, 8, 128], FP8)
            nc.gpsimd.dma_start(out=W8D[:], in_=d_W8D[:])

            # tail-only tensors; DMAs drip-fed on sync from inside the loop
            WhT = cpool.tile([128, 8, H], BF16)
            bh = cpool.tile([1, H], BF16)
            WlT = cpool.tile([128, 8, E], BF16)
            bl = cpool.tile([128, 2], F32)
            bhT = cpool.tile([128, 8], F32)
            cbT = cpool.tile([128, 3], F32)
            cw = [cpool.tile([128, KS[i], 2, NF], BF16, tag=f"cw{i}",
                             name=f"cw{i}t") for i in range(3)]
            WcT = cpool.tile([128, 3, TYPE_NUM], BF16)
            bc = cpool.tile([TYPE_NUM, 1], F32)
            tail_dmas = [(WhT[:, kc, :], d_WhT[:, kc, :]) for kc in range(8)]
            tail_dmas += [(bh[:], d_bh[:]), (WlT[:], d_WlT[:]),
                          (bl[:], d_bl[:]), (bhT[:], d_bhT[:]),
                          (cbT[:], d_cbT[:])]
            tail_dmas += [(cw[i][:], d_cw[i][:]) for i in range(3)]
            tail_dmas += [(WcT[:], d_WcT[:]), (bc[:], d_bc[:])]

            ones = cpool.tile([1, max(n_pad, C)], BF16)
            nc.vector.memset(ones[:], 1.0)
            onesP = cpool.tile([n_pad, 1], BF16)
            nc.vector.memset(onesP[:], 1.0)

            # dense broadcast materializations on DVE (doubling copies):
            # qbc[p, ec, t, c] = qT[p, ec, c]; ctxbc[p, ec, t, c] = ctxT[.., t]
            qbc = cpool.tile([128, 2, 8, C], BF16)
            nc.vector.tensor_copy(qbc[:, :, 0, :], qT[:])
            nc.vector.tensor_copy(qbc[:, :, 1, :], qbc[:, :, 0, :])
            nc.vector.tensor_copy(qbc[:, :, 2:4, :], qbc[:, :, 0:2, :])
            nc.vector.tensor_copy(qbc[:, :, 4:8, :], qbc[:, :, 0:4, :])
            ctxbc = cpool.tile([128, 2, n_pad, C], BF16)

            def build_ctxbc(tsl):
                nc.vector.tensor_copy(ctxbc[:, :, tsl, 0], ctxT[:, :, tsl])
                w = 1
                while w < C:
                    nc.vector.tensor_copy(ctxbc[:, :, tsl, w:2 * w],
                                          ctxbc[:, :, tsl, 0:w])
                    w *= 2

            # build 2 tiles worth now; the rest just-ahead inside the loop
            # (the DVE queue is in-order, so late chunks must be emitted
            # after the earlier tiles' feature ops, not before the loop)
            cuts = list(range(16, n_pad, 16)) + [n_pad]
            build_ctxbc(slice(0, min(16, n_pad)))
            ctxbc_stages = [slice(a, b) for a, b in zip(cuts, cuts[1:])]

            # PE warm-up burst on junk data: keeps the HAM activity window
            # busy while the first DMAs land so phase 0 runs at full clock.
            junk = cpool.tile([128, 512], BF16)
            nc.vector.memset(junk[:], 0.5)
            psj = ps_sm.tile([128, 512], F32, tag="sm")
            for wi in range(14):
                nc.tensor.matmul(psj[:], junk[:, 0:128], junk[:],
                                 start=(wi == 0), stop=(wi == 13))

            def junk_burst(n):
                # PE-idle bridge: enough matmul activity to stop the HAM
                # clock gate from re-throttling during serial scalar/vector
                # sections. Uses the (then idle) ps_main pool.
                pj = ps_main.tile([128, 2, 512], F32, tag="P")
                for wi in range(n):
                    nc.tensor.matmul(pj[:, 0, :], junk[:, 0:128], junk[:],
                                     start=(wi == 0), stop=(wi == n - 1))

            # ---- phase 0: A = q @ W1.T ; B = ctx @ W2.T + b_h (all x64) ---
            # A rows (c: 0..63) and B rows (t: 64..71, rewritten per rt via
            # SBUF->SBUF DMA) stack into the two alternating AB stationaries.
            AB0 = cpool.tile([C + 8, H], BF16, tag="AB0")
            AB1 = cpool.tile([C + 8, H], BF16, tag="AB1")
            B_T = cpool.tile([n_pad, H], BF16)
            for jn in range(2):
                jsl = slice(jn * 512, (jn + 1) * 512)
                psA = ps_sm.tile([C, 512], F32, tag="sm")
                nc.tensor.matmul(psA[:], qT8[:], W8A[:, :, jsl],
                                 start=True, stop=True, perf_mode=DR)
                nc.scalar.copy(AB0[0:C, jsl], psA[:])
                nc.scalar.copy(AB1[0:C, jsl], psA[:])
            for jn in range(2):
                jsl = slice(jn * 512, (jn + 1) * 512)
                psB = ps_sm.tile([n_pad16, 512], F32, tag="sm")
                nc.tensor.matmul(psB[:], ctxT8[:], W8B[:, :, jsl],
                                 start=True, stop=False, perf_mode=DR)
                nc.tensor.matmul(psB[0:n_pad, :], ones[:, :n_pad],
                                 bh64[:, jsl], start=False, stop=True,
                                 skip_group_check=True)
                nc.scalar.copy(B_T[:, jsl], psB[0:n_pad, :])

            if stage < 2:
                nc.gpsimd.dma_start(out=d_out[:], in_=B_T[0:TYPE_NUM, 0])

            junk_burst(14)  # bridge PE over the phase0 -> rt0 dependency gap

            # ---- phase 1: scores over (c, active t) -----------------------
            scoresTT = spool.tile([n_pad, C], F32)
            if stage >= 2:
                ab_tiles = (AB0, AB1)
                tail_fed = 0
                for rt in range(R):
                    AB = ab_tiles[rt % 2]
                    # stationary B rows for this tile -> partitions 64..71
                    nc.sync.dma_start(
                        out=AB[C:C + 8, :],
                        in_=B_T[rt * 8:(rt + 1) * 8, :])
                    if rt >= 2:
                        # drip-feed tail-weight DMAs (3 per rt) on sync
                        for _ in range(3):
                            if tail_fed < len(tail_dmas):
                                o, i_ = tail_dmas[tail_fed]
                                nc.sync.dma_start(out=o, in_=i_)
                                tail_fed += 1
                    ftC = ftpool.tile([128, 2, 8, C], FP8, tag="ftC")
                    ftD = ftpool.tile([128, 2, 8, C], FP8, tag="ftD")
                    sc_t = ftpool.tile([128, 2, 8, C], BF16, tag="sc_t")
                    for ec in range(2):
                        bq = qbc[:, ec]
                        bcx = ctxbc[:, ec, rt * 8:(rt + 1) * 8, :]
                        nc.vector.tensor_sub(sc_t[:, ec], bq, bcx)
                        nc.vector.scalar_tensor_tensor(
                            ftC[:, ec], sc_t[:, ec], -1.0, sc_t[:, ec],
                            op0=ALU.mult, op1=ALU.max)
                        nc.vector.tensor_mul(ftD[:, ec], bq, bcx)
                    if ctxbc_stages:
                        build_ctxbc(ctxbc_stages.pop(0))
                    # MM order per pair keeps fp8-DR matmuls contiguous (a
                    # bf16->DR mode switch costs ~190ns): 4 DR mains + the
                    # previous pair's DR score matmul, then the 2 bf16
                    # indicator matmuls at the end.
                    S = ps_sm.tile([1, 512], F32, tag="sm")
                    ths = []
                    for jp in range(4):  # pairs of 128-wide hidden chunks
                        P2 = ps_main.tile([128, 2, 512], F32, tag="P")
                        TH2 = thpool.tile([128, 2, 512], FP8, tag="TH")
                        jc0, jc1 = jp * 2, jp * 2 + 1
                        jsl0 = slice(jc0 * 128, (jc0 + 1) * 128)
                        jsl1 = slice(jc1 * 128, (jc1 + 1) * 128)
                        nc.tensor.matmul(P2[:, 0, :], W8C[:, :, jc0, :],
                                         ftC[:], start=True, stop=False,
                                         perf_mode=DR)
                        nc.tensor.matmul(P2[:, 0, :], W8D[:, :, jc0, :],
                                         ftD[:], start=False, stop=False,
                                         perf_mode=DR)
                        if jp > 0:
                            nc.tensor.matmul(S[:], Wv8[:, :, jp - 1:jp],
                                             ths[jp - 1][:], start=(jp == 1),
                                             stop=False, perf_mode=DR,
                                             skip_group_check=True)
                        nc.tensor.matmul(P2[:, 1, :], W8C[:, :, jc1, :],
                                         ftC[:], start=True, stop=False,
                                         perf_mode=DR)
                        nc.tensor.matmul(P2[:, 1, :], W8D[:, :, jc1, :],
                                         ftD[:], start=False, stop=False,
                                         perf_mode=DR)
                        nc.tensor.matmul(P2[:, 0, :], AB[:, jsl0], IndAB[:],
                                         start=False, stop=True,
                                         skip_group_check=True)
                        nc.tensor.matmul(P2[:, 1, :], AB[:, jsl1], IndAB[:],
                                         start=False, stop=True,
                                         skip_group_check=True)
                        nc.scalar.activation(TH2[:], P2[:], AF.Tanh,
                                             scale=1.0 / WS)
                        ths.append(TH2)
                    nc.tensor.matmul(S[:], Wv8[:, :, 3:4], ths[3][:],
                                     start=False, stop=True, perf_mode=DR,
                                     skip_group_check=True)
                    S_sb = thpool.tile([1, 512], F32, tag="S_sb")
                    nc.vector.tensor_scalar_mul(S_sb[:], S[:], 1.0 / WS)
                    # partition-scatter 1x512 -> 8 t-rows of scoresTT
                    nc.gpsimd.dma_start(
                        out=scoresTT[rt * 8:(rt + 1) * 8, :],
                        in_=S_sb[0:1, :].rearrange("p (t c) -> p t c", c=C))
                while tail_fed < len(tail_dmas):
                    o, i_ = tail_dmas[tail_fed]
                    nc.sync.dma_start(out=o, in_=i_)
                    tail_fed += 1
            if stage == 2:
                nc.sync.dma_start(out=d_out[:], in_=scoresTT[0:TYPE_NUM, 0])

            # ---- masked softmax + g = attn @ ctx --------------------------
            if stage >= 3:
                # scores are O(1) so exp() is safe without max-subtraction;
                # masked positions are -1e10 -> exp = 0. Everything runs in
                # the native [t, c] layout (t on partitions): the sum over t
                # and the 1/sum broadcast are two tiny PE matmuls, and the
                # gT matmuls consume attnT [t, c] directly -- no transposes.
                junk_burst(8)
                exT = spool.tile([n_pad, C], BF16)
                nl = n_pad - 8  # rows done before the last tile's scatter
                if nl > 0:
                    nc.vector.tensor_add(scoresTT[0:nl, :], scoresTT[0:nl, :],
                                         maskaddT[0:nl, :])
                    nc.scalar.activation(exT[0:nl, :], scoresTT[0:nl, :],
                                         AF.Exp, scale=1.0)
                nc.vector.tensor_add(scoresTT[nl:n_pad, :],
                                     scoresTT[nl:n_pad, :],
                                     maskaddT[nl:n_pad, :])
                nc.scalar.activation(exT[nl:n_pad, :], scoresTT[nl:n_pad, :],
                                     AF.Exp, scale=1.0)
                se_ps = ps_sm.tile([1, C], F32, tag="sm")
                nc.tensor.matmul(se_ps[:], onesP[:, 0:1], exT[:],
                                 start=True, stop=True)
                rse = spool.tile([1, C], BF16)
                with nc.allow_low_precision(reason="bf16 1/sum is plenty"):
                    nc.vector.reciprocal(rse[:], se_ps[:])
                rse_ps = ps_sm.tile([n_pad, C], F32, tag="sm")
                nc.tensor.matmul(rse_ps[:], ones[:, :n_pad], rse[:],
                                 start=True, stop=True)
                attnT = spool.tile([n_pad, C], BF16)
                nc.vector.tensor_mul(attnT[:], exT[:], rse_ps[:])
                junk_burst(4)
                # gT[e, c] = (ctx.T @ attn.T)[e, c] -- direct, no transposes
                gT = spool.tile([128, 2, C], BF16)
                for ec in range(2):
                    gT_ps = ps_sm.tile([128, C], F32, tag="sm")
                    nc.tensor.matmul(gT_ps[:],
                                     ctxa[:, ec * 128:(ec + 1) * 128],
                                     attnT[:], start=True, stop=True)
                    nc.scalar.copy(gT[:, ec, :], gT_ps[:])
            if stage == 3:
                nc.gpsimd.dma_start(out=d_out[:], in_=gT[0:TYPE_NUM, 0, 0])

            # ---- phase 2: h2 = tanh([q|g|,|q-g|,q*g] @ Wh.T + bh) ---------
            if stage >= 4:
                junk_burst(4)
                f2C = spool.tile([128, 2, C], BF16)
                f2D = spool.tile([128, 2, C], BF16)
                for ec in range(2):
                    nc.vector.tensor_sub(f2C[:, ec], qT[:, ec, :], gT[:, ec, :])
                    nc.vector.scalar_tensor_tensor(
                        f2C[:, ec], f2C[:, ec], -1.0, f2C[:, ec],
                        op0=ALU.mult, op1=ALU.max)
                    nc.vector.tensor_mul(f2D[:, ec], qT[:, ec, :], gT[:, ec, :])
                h2T = spool.tile([128, 8, C], BF16)
                for jc in range(8):
                    jsl = slice(jc * 128, (jc + 1) * 128)
                    H2 = ps_sm.tile([128, C], F32, tag="sm")
                    for mi, rhs_t in enumerate((qT[:, 0, :], qT[:, 1, :],
                                                gT[:, 0, :], gT[:, 1, :],
                                                f2C[:, 0, :], f2C[:, 1, :],
                                                f2D[:, 0, :], f2D[:, 1, :])):
                        nc.tensor.matmul(H2[:], WhT[:, mi, jsl], rhs_t,
                                         start=(mi == 0), stop=(mi == 7))
                    nc.scalar.activation(h2T[:, jc, :], H2[:], AF.Tanh,
                                         bias=bhT[:, jc:jc + 1], scale=1.0)

                # x.T = W_lin @ h2 : [e, c], e-major for the convs
                xT = spool.tile([128, 2, C], BF16)
                for ec2 in range(2):
                    X = ps_sm.tile([128, C], F32, tag="sm")
                    for jc in range(8):
                        nc.tensor.matmul(
                            X[:], WlT[:, jc, ec2 * 128:(ec2 + 1) * 128],
                            h2T[:, jc, :], start=(jc == 0), stop=(jc == 7))
                    nc.scalar.activation(xT[:, ec2, :], X[:], AF.Identity,
                                         bias=bl[:, ec2:ec2 + 1], scale=1.0)

                # convs + maxpool; conv bias commutes with max over
                # positions, so it folds into the relu bias afterwards
                pooled_raw = spool.tile([NF, 3], F32)
                for i in range(3):
                    ki = KS[i]
                    oi = C - ki + 1
                    Y = ps_sm.tile([NF, oi], F32, tag="sm")
                    nmm = 2 * ki
                    mm = 0
                    for dk in range(ki):
                        for ec2 in range(2):
                            nc.tensor.matmul(Y[:], cw[i][:, dk, ec2, :],
                                             xT[:, ec2, dk:dk + oi],
                                             start=(mm == 0),
                                             stop=(mm == nmm - 1))
                            mm += 1
                    nc.vector.tensor_reduce(pooled_raw[:, i:i + 1], Y[:],
                                            axis=mybir.AxisListType.X,
                                            op=ALU.max)
                pooled = spool.tile([NF, 3], BF16)
                for i in range(3):
                    nc.scalar.activation(pooled[:, i:i + 1],
                                         pooled_raw[:, i:i + 1], AF.Relu,
                                         bias=cbT[:, i:i + 1], scale=1.0)

                # final linear: out = W_cnn @ cnn + b_cnn
                O = ps_sm.tile([TYPE_NUM, 1], F32, tag="sm")
                for i in range(3):
                    nc.tensor.matmul(O[:], WcT[:, i, :], pooled[:, i:i + 1],
                                     start=(i == 0), stop=(i == 2))
                out_sb = spool.tile([TYPE_NUM, 1], F32)
                nc.scalar.activation(out_sb[:], O[:], AF.Identity, bias=bc[:],
                                     scale=1.0)
                nc.sync.dma_start(out=d_out[:], in_=out_sb[:, 0])

    nc.compile()
    nc.m = get_hw_module(nc.m)
    return nc


def _prep_inputs(query, context, mask, W_hidden, b_hidden, W_v, b_v,
                 W_lin, b_lin, conv_w0, conv_b0, conv_w1, conv_b1,
                 conv_w2, conv_b2, W_cnn, b_cnn):
    """Host-side layout prep. Returns (n_pad, per_core_maps)."""
    f32 = np.float32
    mask = np.asarray(mask)
    n_act = mask.sum(1)
    if n_act.min() == 0:
        # degenerate: keep every position, mask on device via maskadd
        idxs = [np.arange(T) for _ in range(B)]
        n_pad = T
        mads = [np.where(mask[b] < 1, NEG, 0.0).astype(f32) for b in range(B)]
    else:
        n_pad = max(8, int(-(-int(n_act.max()) // 8) * 8))
        idxs, mads = [], []
        for b in range(B):
            idx = np.nonzero(mask[b])[0]
            ma = np.full(n_pad, NEG, f32)
            ma[:len(idx)] = 0.0
            idx = np.concatenate([idx, np.zeros(n_pad - len(idx), np.int64)])
            idxs.append(idx)
            mads.append(ma)
    n_pad16 = -(-n_pad // 16) * 16

    bf = bfloat16
    f8 = float8_e4m3
    Wh = np.asarray(W_hidden, f32)
    # W chunk c (of 8) = rows/cols [c*128:(c+1)*128] of the k=1024 dim.
    # WhT[p, kc, j] = Wh[j, kc*128 + p]
    WhT = np.ascontiguousarray(Wh.T).reshape(8, 128, H).transpose(1, 0, 2)
    WhTs = WhT * WS
    # fp8 pair tensors: [ki, pair(2), ...]
    W8A = np.ascontiguousarray(WhTs[:, 0:2, :])            # k chunks 0,1 (q)
    W8B = np.ascontiguousarray(WhTs[:, 2:4, :])            # k chunks 2,3 (ctx)
    W8C = np.ascontiguousarray(
        WhTs[:, 4:6, :].reshape(128, 2, 8, 128))           # |q-ctx|
    W8D = np.ascontiguousarray(
        WhTs[:, 6:8, :].reshape(128, 2, 8, 128))           # q*ctx
    Wv8 = np.zeros((128, 2, 16), f32)
    Wv8[:, :, 0:4] = (np.asarray(W_v, f32)[0] * WS).reshape(4, 2, 128) \
        .transpose(2, 1, 0)

    IndAB = np.concatenate([
        np.tile(np.eye(C, dtype=f32), (1, 8)),
        np.kron(np.eye(8, dtype=f32), np.ones((1, C), f32)),
    ], axis=0)

    query = np.asarray(query, f32)
    qTf = np.ascontiguousarray(query.T.reshape(2, 128, C).transpose(1, 0, 2))
    shared = {
        "qT": qTf.astype(bf),
        "qT8": qTf.astype(f8),
        "W8A": W8A.astype(f8),
        "W8B": W8B.astype(f8),
        "W8C": W8C.astype(f8),
        "W8D": W8D.astype(f8),
        "Wv8": Wv8.astype(f8),
        "IndAB": IndAB.astype(bf),
        "bh64": (np.asarray(b_hidden, f32) * WS).reshape(1, H).astype(bf),
        "WhT": np.ascontiguousarray(WhT).astype(bf),
        "bh": np.asarray(b_hidden, f32).reshape(1, H).astype(bf),
        "WlT": np.ascontiguousarray(
            np.asarray(W_lin, f32).T.reshape(8, 128, E).transpose(1, 0, 2)
        ).astype(bf),
        "bl": np.ascontiguousarray(
            np.asarray(b_lin, f32).reshape(2, 128).T).astype(f32),
        "bhT": np.ascontiguousarray(
            np.asarray(b_hidden, f32).reshape(8, 128).T).astype(f32),
        "cbT": np.stack([np.asarray(x, f32) for x in
                         (conv_b0, conv_b1, conv_b2)], axis=1).astype(f32),
        "WcT": np.ascontiguousarray(
            np.asarray(W_cnn, f32).T.reshape(3, 128, TYPE_NUM)
            .transpose(1, 0, 2)).astype(bf),
        "bc": np.asarray(b_cnn, f32).reshape(TYPE_NUM, 1).astype(f32),
    }
    for i, w in enumerate((conv_w0, conv_w1, conv_w2)):
        w = np.asarray(w, f32)  # [NF, E, ki]
        arr = w.transpose(1, 2, 0).reshape(2, 128, KS[i], NF) \
            .transpose(1, 2, 0, 3)  # [128, ki, 2, NF]
        shared[f"cw{i}"] = np.ascontiguousarray(arr).astype(bf)

    context = np.asarray(context, f32)
    per_core = []
    for b in range(B):
        ctx_act = context[b][idxs[b]]  # [n_pad, E]
        ctx_act = ctx_act * (mads[b] == 0.0)[:, None]  # zero padded rows
        ctxT = np.ascontiguousarray(
            ctx_act.T.reshape(2, 128, n_pad).transpose(1, 0, 2))
        ctxT8 = np.zeros((128, 2, n_pad16), f32)
        ctxT8[:, :, :n_pad] = ctxT
        per_core.append({
            "ctx": np.ascontiguousarray(ctx_act).astype(bf),
            "ctxT": ctxT.astype(bf),
            "ctxT8": ctxT8.astype(f8),
            "maskadd": np.tile(mads[b][:, None], (1, C)).astype(f32),
            **shared,
        })
    return n_pad, per_core


def kernel(**inputs):
    global LAST_EXEC_NS
    n_pad, per_core = _prep_inputs(**inputs)
    key = (n_pad, os.environ.get("KSTAGE", "99"))
    if key not in _CACHE:
        _CACHE[key] = _build_program(n_pad)
    nc = _CACHE[key]
    res = run_bass_kernel_spmd(nc, per_core, list(range(NUM_CORES)),
                               trace=TRACE)
    LAST_EXEC_NS = res.exec_time_ns
    out = np.stack([res.results[i]["out"] for i in range(NUM_CORES)])
    return out.astype(np.float32)
